# revision 27
# baseline (speedup 1.0000x reference)
"""Trainium2 Bass kernel for AdaptivePhysicallyConstrainedAttention.

Model (see problem reference): top-k-masked dense attention + residual + LayerNorm.
  mask  = top-3 columns of softmax(band_importance) -> additive -inf bias
  q,k,v = x @ W{q,k,v}.T + b        (B=4, L=2048, D=1024, H=16, hd=64)
  attn  = softmax(q k^T / 8 + bias) v ;  out = LN(x + attn @ Wo.T + bo) * gamma + beta

Sharding: 8 cores = (batch 4) x (query-halves 2). Each core computes K/V for its
full batch (duplicated within the pair) and attends its 1024 query rows — no
collectives. Host prep: top-k mask (tiny), weight transposes, bf16 casts, and a
per-core column permutation of x^T so every core's own query rows sit first
(keeps the graph SPMD-uniform).

On-device structure:
  - scores computed transposed (S^T = K Q^T) so the column mask is a
    per-partition activation bias and exp output feeds the AV matmul as lhsT
  - exp runs 1024-wide out of two PSUM banks (ScalarE is the critical engine;
    wide activations amortize its access latency)
  - V stored [k, head, 65] with a ones column -> AV matmul also produces the
    softmax denominator; normalization is a per-partition scale afterwards
  - all projection work is emitted as background chunks popped between
    attention S-steps so ScalarE never starves
  - matmuls in bf16 (fp32 accumulation), everything else fp32
"""

import sys

if "/opt/trn_rl_repo" not in sys.path:
    sys.path.insert(0, "/opt/trn_rl_repo")

from collections import deque

import numpy as np
import ml_dtypes

import concourse.bass as bass  # noqa: F401  (registers engines)
import concourse.tile as tile
from concourse import bacc, mybir
from concourse.bass_utils import run_bass_kernel_spmd
from concourse.masks import make_identity

BF16 = mybir.dt.bfloat16
F32 = mybir.dt.float32
AF = mybir.ActivationFunctionType
OP = mybir.AluOpType

B, L, D, H, HD = 4, 2048, 1024, 16, 64
LQ = L // 2  # query rows per core
P = 128
NCORES = 8
TOPK = 3
SCALE = 1.0 / 8.0
MASK_BIAS = -10000.0
LN_EPS = 1e-5

NIT = D // P        # 8   contraction tiles over D
NOT = D // P        # 8   output tiles over D
NKT = L // P        # 16  key tiles
NQS = LQ // P       # 8   query subtiles
NLT = LQ // P       # 8   own-row tiles


def build_nc():
    nc = bacc.Bacc(None, target_bir_lowering=False, debug=False)

    xT = nc.declare_dram_parameter("xT", [D, L], BF16, isOutput=False)
    xres = nc.declare_dram_parameter("xres", [LQ, D], F32, isOutput=False)
    wqT = nc.declare_dram_parameter("wqT", [D, D], BF16, isOutput=False)
    wkT = nc.declare_dram_parameter("wkT", [D, D], BF16, isOutput=False)
    wvT = nc.declare_dram_parameter("wvT", [D, D], BF16, isOutput=False)
    woT = nc.declare_dram_parameter("woT", [D, D], BF16, isOutput=False)
    # packed small consts: cols 0:16 bias_k, 16:24 bq, 24:32 bk
    cpack = nc.declare_dram_parameter("cpack", [P, 32], F32, isOutput=False)
    bvb = nc.declare_dram_parameter("bvb", [P, D], BF16, isOutput=False)
    # packed gamma/beta broadcast: cols 0:D gamma, D:2D beta
    gbeta = nc.declare_dram_parameter("gbeta", [P, 2 * D], F32, isOutput=False)
    out = nc.declare_dram_parameter("out", [LQ, D], F32, isOutput=True)

    with tile.TileContext(nc) as tc:
        with (
            tc.tile_pool(name="const", bufs=1) as constp,
            tc.tile_pool(name="big", bufs=1) as bigp,
            tc.tile_pool(name="wstream", bufs=2) as wsp,
            tc.tile_pool(name="ps", bufs=2, space="PSUM") as psp,
            tc.tile_pool(name="shps", bufs=2, space="PSUM") as shpsp,
            tc.tile_pool(name="ctxps", bufs=2, space="PSUM") as ctxpsp,
            tc.tile_pool(name="pt", bufs=34) as ptp,
            tc.tile_pool(name="qkstr", bufs=3) as qkp,
            tc.tile_pool(name="small", bufs=4) as smallp,
            tc.tile_pool(name="io", bufs=2) as iop,
        ):
            # ---- resident tensors; DMAs emitted in startup-priority order ----
            xT_sb = bigp.tile([P, NIT, L], BF16, tag="xT")
            nc.sync.dma_start(
                out=xT_sb[:, :, 0:512],
                in_=xT[:, 0:512].rearrange("(t p) l -> p t l", p=P),
            )

            def qk_dma(ot, w_dram):
                wt = wsp.tile([P, NIT, P], BF16, tag="wqk")
                nc.sync.dma_start(
                    out=wt[:],
                    in_=w_dram[:, ot * P : (ot + 1) * P].rearrange(
                        "(t p) o -> p t o", p=P
                    ),
                )
                return wt

            wq0 = qk_dma(0, wqT)
            wk0 = qk_dma(0, wkT)
            cp_sb = constp.tile([P, 32], F32, tag="cpack")
            nc.sync.dma_start(out=cp_sb[:], in_=cpack[:, :])
            biask_sb = cp_sb[:, 0:16]
            bq_sb = cp_sb[:, 16:24]
            bk_sb = cp_sb[:, 24:32]
            for lch in range(1, 4):
                nc.sync.dma_start(
                    out=xT_sb[:, :, lch * 512 : (lch + 1) * 512],
                    in_=xT[:, lch * 512 : (lch + 1) * 512].rearrange(
                        "(t p) l -> p t l", p=P
                    ),
                )

            def v_dma(og):
                wt = wsp.tile([P, NIT, 512], BF16, tag="wv")
                nc.sync.dma_start(
                    out=wt[:],
                    in_=wvT[:, og * 512 : (og + 1) * 512].rearrange(
                        "(t p) o -> p t o", p=P
                    ),
                )
                return wt

            wv0 = v_dma(0)
            bvb_sb = constp.tile([P, D], BF16, tag="bvb")
            nc.sync.dma_start(out=bvb_sb[:], in_=bvb[:, :])

            eps_sb = constp.tile([P, 1], F32, tag="eps")
            nc.vector.memset(eps_sb[:], LN_EPS)
            ident = constp.tile([P, P], BF16, tag="ident")
            make_identity(nc, ident[:])

            v_sb = bigp.tile([P, NKT, H, HD + 1], BF16, tag="v")
            ctxT_sb = bigp.tile([P, NIT, LQ], BF16, tag="ctxT")

            # ones column of the augmented V (softmax denominator trick)
            nc.vector.memset(v_sb[:, :, :, HD : HD + 1], 1.0)

            # ---- projection chunk emitters (each ~1.7us of PE work) ----
            def qk_chunk(wt, ot, bias_sb, dst_tile, lc):
                ps = shpsp.tile([P, 512], F32, tag="shps")
                for it in range(NIT):
                    nc.tensor.matmul(
                        ps[:],
                        wt[:, it, :],
                        xT_sb[:, it, lc * 512 : (lc + 1) * 512],
                        start=(it == 0),
                        stop=(it == NIT - 1),
                    )
                nc.vector.tensor_scalar(
                    out=dst_tile[:, lc * 512 : (lc + 1) * 512],
                    in0=ps[:],
                    scalar1=bias_sb[:, ot : ot + 1],
                    scalar2=None,
                    op0=OP.add,
                )

            qk_tiles = {}

            def alloc_qk(ot):
                qt = qkp.tile([P, LQ], BF16, tag="qstr")
                ktl = qkp.tile([P, L], BF16, tag="kstr")
                qk_tiles[ot] = (qt, ktl)
                return qt, ktl

            def v_chunk(wt, og, lt):
                ps = shpsp.tile([P, 512], F32, tag="shps")
                for it in range(NIT):
                    nc.tensor.matmul(
                        ps[:],
                        xT_sb[:, it, lt * P : (lt + 1) * P],
                        wt[:, it, :],
                        start=(it == 0),
                        stop=(it == NIT - 1),
                    )
                nc.vector.tensor_tensor(
                    out=v_sb[:, lt, 8 * og : 8 * og + 8, 0:HD],
                    in0=ps[:].rearrange("p (h d) -> p h d", h=8),
                    in1=bvb_sb[:, og * 512 : (og + 1) * 512].rearrange(
                        "p (h d) -> p h d", h=8
                    ),
                    op=OP.add,
                )

            bg_urgent = deque()
            bg = deque()
            _step = [0]

            def bg_pop(force=False):
                # urgent chunks drain greedily; paced chunks every other slot
                if bg_urgent:
                    bg_urgent.popleft()()
                    return
                _step[0] += 1
                if bg and (force or _step[0] % 2 == 0):
                    bg.popleft()()

            # ---- attention: one 1024-wide q chunk per head ----
            # software-pipelined across heads: head h's AV/normalize phase is
            # interleaved with head h+1's S/exp phase so PE always has S work
            # while DVE normalizes and ACT streams exps.

            def s_phase(h):
                po = (h % 2) * HD
                ot = h // 2
                p_tiles = []
                for kt in range(NKT):
                    sps = psp.tile([P, 1024], F32, tag="sps")
                    for qh in range(2):
                        nc.tensor.matmul(
                            sps[:, qh * 512 : (qh + 1) * 512],
                            kT_sb[po : po + HD, ot, kt * P : (kt + 1) * P],
                            qT_sb[po : po + HD, ot, qh * 512 : (qh + 1) * 512],
                            start=True,
                            stop=True,
                        )
                    pt = ptp.tile([P, 1024], BF16, tag="pt")
                    nc.scalar.activation(
                        out=pt[:],
                        in_=sps[:],
                        func=AF.Exp,
                        bias=biask_sb[:, kt : kt + 1],
                        scale=SCALE,
                    )
                    p_tiles.append(pt)
                    bg_pop()
                    yield

            def av_phase(h, p_tiles):
                po = (h % 2) * HD
                ot = h // 2
                tr_ps = shpsp.tile([HD, 1024], BF16, tag="shps")
                for qs in range(NQS):
                    ctx_ps = ctxpsp.tile([P, HD + 1], F32, tag="ctx")
                    for kt in range(NKT):
                        nc.tensor.matmul(
                            ctx_ps[:],
                            p_tiles[kt][:, qs * P : (qs + 1) * P],
                            v_sb[:, kt, h, :],
                            start=(kt == 0),
                            stop=(kt == NKT - 1),
                        )
                    den = smallp.tile([P, 1], F32, tag="den")
                    nc.vector.reciprocal(den[:], ctx_ps[:, HD : HD + 1])
                    cn = smallp.tile([P, HD], BF16, tag="cn")
                    nc.vector.tensor_scalar(
                        out=cn[:],
                        in0=ctx_ps[:, 0:HD],
                        scalar1=den[:, 0:1],
                        scalar2=None,
                        op0=OP.mult,
                    )
                    nc.tensor.transpose(
                        tr_ps[:, qs * P : (qs + 1) * P], cn[:], ident[:]
                    )
                    bg_pop()
                    yield
                nc.vector.tensor_copy(
                    out=ctxT_sb[po : po + HD, ot, :], in_=tr_ps[:]
                )

            def run_interleaved(gens_weights):
                """Round-robin generators: (gen, steps_per_turn)."""
                live = [[g, w] for g, w in gens_weights]
                while live:
                    for gw in list(live):
                        g, w = gw
                        for _ in range(w):
                            try:
                                next(g)
                            except StopIteration:
                                live.remove(gw)
                                break

            # ---- startup: eagerly project what head 0 needs first ----
            qt0, ktl0 = alloc_qk(0)
            for lc in range(2):
                qk_chunk(wq0, 0, bq_sb, qt0, lc)
            for lc in range(4):
                qk_chunk(wk0, 0, bk_sb, ktl0, lc)
            for lt in range(4):
                v_chunk(wv0, 0, lt)

            wv1 = [None]

            def queue_group(ot):
                # work queued at group ot, popped during its heads' S-steps
                if ot == 0:
                    for lt in range(4, NKT):
                        bg_urgent.append(lambda lt=lt: v_chunk(wv0, 0, lt))
                if ot < NOT - 1:
                    wtq = qk_dma(ot + 1, wqT)
                    wtk = qk_dma(ot + 1, wkT)
                    qt, ktl = alloc_qk(ot + 1)
                    for lc in range(2):
                        bg.append(
                            lambda wt=wtq, ot=ot, lc=lc, qt=qt: qk_chunk(
                                wt, ot + 1, bq_sb, qt, lc
                            )
                        )
                    for lc in range(4):
                        bg.append(
                            lambda wt=wtk, ot=ot, lc=lc, ktl=ktl: qk_chunk(
                                wt, ot + 1, bk_sb, ktl, lc
                            )
                        )
                if ot == 1:
                    wv1[0] = v_dma(1)
                if 1 <= ot <= 4:
                    for lt in range(4 * (ot - 1), 4 * ot):
                        bg.append(lambda lt=lt: v_chunk(wv1[0], 1, lt))

            def s_phase_collect(h):
                gen = s_phase(h)
                tiles_ref = []

                def stepper():
                    frame_locals = None
                    for _ in gen:
                        yield
                    # generator finished; grab its p_tiles via closure below

                return gen

            # run the pipeline: S(0); then for h: interleave AV(h-1) with S(h)
            class HeadState:
                pass

            def make_s(h):
                st = HeadState()
                st.tiles = []
                po = (h % 2) * HD
                ot = h // 2

                def gen():
                    qt, ktl = qk_tiles[ot]
                    for kt in range(NKT):
                        sps = psp.tile([P, 1024], F32, tag="sps")
                        for qh in range(2):
                            nc.tensor.matmul(
                                sps[:, qh * 512 : (qh + 1) * 512],
                                ktl[po : po + HD, kt * P : (kt + 1) * P],
                                qt[po : po + HD, qh * 512 : (qh + 1) * 512],
                                start=True,
                                stop=True,
                            )
                        pt = ptp.tile([P, 1024], BF16, tag="pt")
                        nc.scalar.activation(
                            out=pt[:],
                            in_=sps[:],
                            func=AF.Exp,
                            bias=biask_sb[:, kt : kt + 1],
                            scale=SCALE,
                        )
                        st.tiles.append(pt)
                        bg_pop()
                        yield

                st.gen = gen()
                return st

            queue_group(0)
            st = make_s(0)
            for _ in st.gen:
                pass
            for h in range(1, H):
                if h % 2 == 0:
                    queue_group(h // 2)
                st_next = make_s(h)
                run_interleaved([(av_phase(h - 1, st.tiles), 1), (st_next.gen, 2)])
                st = st_next
            for _ in av_phase(H - 1, st.tiles):
                pass
            while bg_urgent:
                bg_urgent.popleft()()
            while bg:
                bg.popleft()()

            # ---- output projection + residual + layernorm ----
            gb_sb = bigp.tile([P, 2 * D], F32, tag="xT")  # reuses xT's slot
            nc.sync.dma_start(out=gb_sb[:], in_=gbeta[:, :])
            gamb_sb = gb_sb[:, 0:D]
            betb_sb = gb_sb[:, D : 2 * D]
            wo_tiles = []
            for oc in range(2):
                wt = wsp.tile([P, NIT, 512], BF16, tag="wv")  # reuses wv slots
                nc.sync.dma_start(
                    out=wt[:],
                    in_=woT[:, oc * 512 : (oc + 1) * 512].rearrange(
                        "(t p) o -> p t o", p=P
                    ),
                )
                wo_tiles.append(wt)

            for lt in range(NLT):
                xr = iop.tile([P, D], F32, tag="xr")
                nc.sync.dma_start(out=xr[:], in_=xres[lt * P : (lt + 1) * P, :])
                y = iop.tile([P, D], F32, tag="y")
                for oc in range(2):
                    ps = shpsp.tile([P, 512], F32, tag="shps")
                    for it in range(NIT):
                        nc.tensor.matmul(
                            ps[:],
                            ctxT_sb[:, it, lt * P : (lt + 1) * P],
                            wo_tiles[oc][:, it, :],
                            start=(it == 0),
                            stop=(it == NIT - 1),
                        )
                    nc.vector.tensor_tensor(
                        out=y[:, oc * 512 : (oc + 1) * 512],
                        in0=ps[:],
                        in1=xr[:, oc * 512 : (oc + 1) * 512],
                        op=OP.add,
                    )
                stats = smallp.tile([P, 2, 6], F32, tag="stats")
                nc.vector.bn_stats(stats[:, 0, :], y[:, 0:512])
                nc.vector.bn_stats(stats[:, 1, :], y[:, 512:1024])
                mv = smallp.tile([P, 2], F32, tag="mv")
                nc.vector.bn_aggr(mv[:], stats[:])
                std = smallp.tile([P, 1], F32, tag="std")
                nc.scalar.activation(
                    out=std[:], in_=mv[:, 1:2], func=AF.Sqrt, bias=eps_sb[:, 0:1]
                )
                rstd = smallp.tile([P, 1], F32, tag="rstd")
                nc.vector.reciprocal(rstd[:], std[:])
                nmr = smallp.tile([P, 1], F32, tag="nmr")  # -mu * rstd
                nc.vector.tensor_scalar(
                    out=nmr[:],
                    in0=mv[:, 0:1],
                    scalar1=rstd[:, 0:1],
                    scalar2=-1.0,
                    op0=OP.mult,
                    op1=OP.mult,
                )
                yn = iop.tile([P, D], F32, tag="xr")
                # (y - mu) * rstd on the (otherwise idle) scalar engine
                nc.scalar.activation(
                    out=yn[:],
                    in_=y[:],
                    func=AF.Identity,
                    bias=nmr[:, 0:1],
                    scale=rstd[:, 0:1],
                )
                o_sb = iop.tile([P, D], F32, tag="y")
                nc.vector.tensor_tensor(out=o_sb[:], in0=yn[:], in1=gamb_sb, op=OP.mult)
                nc.gpsimd.tensor_tensor(out=o_sb[:], in0=o_sb[:], in1=betb_sb, op=OP.add)
                nc.sync.dma_start(out=out[lt * P : (lt + 1) * P, :], in_=o_sb[:])

    nc.compile()
    return nc


def host_prep(inputs):
    """Shard + lay out the full inputs into 8 per-core in_maps."""
    bf16 = ml_dtypes.bfloat16
    x = np.asarray(inputs["x"], dtype=np.float32)
    bi = np.asarray(inputs["band_importance"], dtype=np.float32)[0]
    idx = np.argpartition(-bi, TOPK)[:TOPK]  # top-k of softmax == top-k of logits
    bias_vec = np.zeros(L, np.float32)
    bias_vec[idx] = MASK_BIAS

    wqTn = np.ascontiguousarray(np.asarray(inputs["Wq"], np.float32).T).astype(bf16)
    wkTn = np.ascontiguousarray(np.asarray(inputs["Wk"], np.float32).T).astype(bf16)
    wvTn = np.ascontiguousarray(np.asarray(inputs["Wv"], np.float32).T).astype(bf16)
    woTn = np.ascontiguousarray(np.asarray(inputs["Wo"], np.float32).T).astype(bf16)
    bq = np.asarray(inputs["bq"], np.float32).reshape(NOT, P).T
    bk = np.asarray(inputs["bk"], np.float32).reshape(NOT, P).T
    bv = np.asarray(inputs["bv"], np.float32)
    bo = np.asarray(inputs["bo"], np.float32)
    gam = np.asarray(inputs["gamma"], np.float32)
    bet = np.asarray(inputs["beta"], np.float32)
    bvb = np.ascontiguousarray(np.broadcast_to(bv, (P, D))).astype(bf16)
    gbeta = np.ascontiguousarray(
        np.concatenate(
            [np.broadcast_to(gam, (P, D)), np.broadcast_to(bet, (P, D))], axis=1
        )
    )

    in_maps = []
    for c in range(NCORES):
        b, hh = c // 2, c % 2
        own = slice(hh * LQ, (hh + 1) * LQ)
        oth = slice((1 - hh) * LQ, (2 - hh) * LQ)
        xTb = x[b].T  # [D, L] view
        xT_c = np.concatenate([xTb[:, own], xTb[:, oth]], axis=1).astype(bf16)
        pb = np.concatenate([bias_vec[own], bias_vec[oth]])
        biask_c = pb.reshape(NKT, P).T
        cpack_c = np.ascontiguousarray(
            np.concatenate([biask_c, bq, bk], axis=1), dtype=np.float32
        )
        xres_c = np.ascontiguousarray(x[b, own]) + bo[None, :]
        in_maps.append(
            {
                "xT": xT_c,
                "xres": xres_c,
                "wqT": wqTn,
                "wkT": wkTn,
                "wvT": wvTn,
                "woT": woTn,
                "cpack": cpack_c,
                "bvb": bvb,
                "gbeta": gbeta,
            }
        )
    return in_maps


def assemble(results):
    out = np.empty((B, L, D), np.float32)
    for c in range(NCORES):
        b, hh = c // 2, c % 2
        out[b, hh * LQ : (hh + 1) * LQ, :] = results[c]["out"]
    return out


_NC_CACHE = None


def kernel(**inputs):
    global _NC_CACHE
    if _NC_CACHE is None:
        _NC_CACHE = build_nc()
    in_maps = host_prep(inputs)
    res = run_bass_kernel_spmd(_NC_CACHE, in_maps, core_ids=list(range(NCORES)))
    return assemble(res.results)


# revision 31
# speedup vs baseline: 7130.4187x; 7130.4187x over previous
"""Trainium2 Bass kernel for AdaptivePhysicallyConstrainedAttention.

Model (see problem reference): top-k-masked dense attention + residual + LayerNorm.
  mask  = top-3 columns of softmax(band_importance) -> additive -inf bias
  q,k,v = x @ W{q,k,v}.T + b        (B=4, L=2048, D=1024, H=16, hd=64)
  attn  = softmax(q k^T / 8 + bias) v ;  out = LN(x + attn @ Wo.T + bo) * gamma + beta

Sharding: 8 cores = (batch 4) x (query-halves 2). Each core computes K/V for its
full batch (duplicated within the pair) and attends its 1024 query rows — no
collectives. Host prep: top-k mask (tiny), weight transposes, bf16 casts, and a
per-core column permutation of x^T so every core's own query rows sit first
(keeps the graph SPMD-uniform).

On-device structure:
  - scores computed transposed (S^T = K Q^T) so the column mask is a
    per-partition activation bias and exp output feeds the AV matmul as lhsT
  - exp runs 1024-wide out of two PSUM banks (ScalarE is the critical engine;
    wide activations amortize its access latency)
  - V stored [k, head, 65] with a ones column -> AV matmul also produces the
    softmax denominator; normalization is a per-partition scale afterwards
  - all projection work is emitted as background chunks popped between
    attention S-steps so ScalarE never starves
  - matmuls in bf16 (fp32 accumulation), everything else fp32
"""

import sys

if "/opt/trn_rl_repo" not in sys.path:
    sys.path.insert(0, "/opt/trn_rl_repo")

from collections import deque

import numpy as np
import ml_dtypes

import concourse.bass as bass  # noqa: F401  (registers engines)
import concourse.tile as tile
from concourse import bacc, mybir
from concourse.bass_utils import run_bass_kernel_spmd
from concourse.masks import make_identity

BF16 = mybir.dt.bfloat16
F32 = mybir.dt.float32
AF = mybir.ActivationFunctionType
OP = mybir.AluOpType

B, L, D, H, HD = 4, 2048, 1024, 16, 64
LQ = L // 2  # query rows per core
P = 128
NCORES = 8
TOPK = 3
SCALE = 1.0 / 8.0
MASK_BIAS = -10000.0
LN_EPS = 1e-5

NIT = D // P        # 8   contraction tiles over D
NOT = D // P        # 8   output tiles over D
NKT = L // P        # 16  key tiles
NQS = LQ // P       # 8   query subtiles
NLT = LQ // P       # 8   own-row tiles


def build_nc():
    nc = bacc.Bacc(None, target_bir_lowering=False, debug=False)

    xT = nc.declare_dram_parameter("xT", [D, L], BF16, isOutput=False)
    xres = nc.declare_dram_parameter("xres", [LQ, D], F32, isOutput=False)
    wqT = nc.declare_dram_parameter("wqT", [D, D], BF16, isOutput=False)
    wkT = nc.declare_dram_parameter("wkT", [D, D], BF16, isOutput=False)
    wvT = nc.declare_dram_parameter("wvT", [D, D], BF16, isOutput=False)
    woT = nc.declare_dram_parameter("woT", [D, D], BF16, isOutput=False)
    # packed small consts: cols 0:16 bias_k, 16:24 bq, 24:32 bk
    cpack = nc.declare_dram_parameter("cpack", [P, 32], F32, isOutput=False)
    bvb = nc.declare_dram_parameter("bvb", [P, D], BF16, isOutput=False)
    # packed gamma/beta broadcast: cols 0:D gamma, D:2D beta
    gbeta = nc.declare_dram_parameter("gbeta", [P, 2 * D], F32, isOutput=False)
    out = nc.declare_dram_parameter("out", [LQ, D], F32, isOutput=True)

    with tile.TileContext(nc) as tc:
        with (
            tc.tile_pool(name="const", bufs=1) as constp,
            tc.tile_pool(name="big", bufs=1) as bigp,
            tc.tile_pool(name="wstream", bufs=2) as wsp,
            tc.tile_pool(name="ps", bufs=2, space="PSUM") as psp,
            tc.tile_pool(name="shps", bufs=2, space="PSUM") as shpsp,
            tc.tile_pool(name="ctxps", bufs=2, space="PSUM") as ctxpsp,
            tc.tile_pool(name="pt", bufs=34) as ptp,
            tc.tile_pool(name="qkstr", bufs=3) as qkp,
            tc.tile_pool(name="small", bufs=4) as smallp,
            tc.tile_pool(name="io", bufs=2) as iop,
        ):
            # ---- resident tensors; DMAs emitted in startup-priority order ----
            xT_sb = bigp.tile([P, NIT, L], BF16, tag="xT")
            nc.sync.dma_start(
                out=xT_sb[:, :, 0:512],
                in_=xT[:, 0:512].rearrange("(t p) l -> p t l", p=P),
            )

            def qk_dma(ot, w_dram):
                wt = wsp.tile([P, NIT, P], BF16, tag="wqk")
                nc.sync.dma_start(
                    out=wt[:],
                    in_=w_dram[:, ot * P : (ot + 1) * P].rearrange(
                        "(t p) o -> p t o", p=P
                    ),
                )
                return wt

            wq0 = qk_dma(0, wqT)
            wk0 = qk_dma(0, wkT)
            cp_sb = constp.tile([P, 32], F32, tag="cpack")
            nc.sync.dma_start(out=cp_sb[:], in_=cpack[:, :])
            biask_sb = cp_sb[:, 0:16]
            bq_sb = cp_sb[:, 16:24]
            bk_sb = cp_sb[:, 24:32]
            for lch in range(1, 4):
                nc.sync.dma_start(
                    out=xT_sb[:, :, lch * 512 : (lch + 1) * 512],
                    in_=xT[:, lch * 512 : (lch + 1) * 512].rearrange(
                        "(t p) l -> p t l", p=P
                    ),
                )

            def v_dma(og):
                wt = wsp.tile([P, NIT, 512], BF16, tag="wv")
                nc.sync.dma_start(
                    out=wt[:],
                    in_=wvT[:, og * 512 : (og + 1) * 512].rearrange(
                        "(t p) o -> p t o", p=P
                    ),
                )
                return wt

            wv0 = v_dma(0)
            bvb_sb = constp.tile([P, D], BF16, tag="bvb")
            nc.sync.dma_start(out=bvb_sb[:], in_=bvb[:, :])

            eps_sb = constp.tile([P, 1], F32, tag="eps")
            nc.vector.memset(eps_sb[:], LN_EPS)
            ident = constp.tile([P, P], BF16, tag="ident")
            make_identity(nc, ident[:])

            v_sb = bigp.tile([P, NKT, H, HD + 1], BF16, tag="v")
            ctxT_sb = bigp.tile([P, NIT, LQ], BF16, tag="ctxT")

            # ones column of the augmented V (softmax denominator trick)
            nc.vector.memset(v_sb[:, :, :, HD : HD + 1], 1.0)

            # ---- projection chunk emitters (each ~1.7us of PE work) ----
            def qk_chunk(wt, ot, bias_sb, dst_tile, lc):
                ps = shpsp.tile([P, 512], F32, tag="shps")
                for it in range(NIT):
                    nc.tensor.matmul(
                        ps[:],
                        wt[:, it, :],
                        xT_sb[:, it, lc * 512 : (lc + 1) * 512],
                        start=(it == 0),
                        stop=(it == NIT - 1),
                    )
                nc.vector.tensor_scalar(
                    out=dst_tile[:, lc * 512 : (lc + 1) * 512],
                    in0=ps[:],
                    scalar1=bias_sb[:, ot : ot + 1],
                    scalar2=None,
                    op0=OP.add,
                )

            qk_tiles = {}

            def alloc_qk(ot):
                qt = qkp.tile([P, LQ], BF16, tag="qstr")
                ktl = qkp.tile([P, L], BF16, tag="kstr")
                qk_tiles[ot] = (qt, ktl)
                return qt, ktl

            def v_chunk(wt, og, lt):
                ps = shpsp.tile([P, 512], F32, tag="shps")
                for it in range(NIT):
                    nc.tensor.matmul(
                        ps[:],
                        xT_sb[:, it, lt * P : (lt + 1) * P],
                        wt[:, it, :],
                        start=(it == 0),
                        stop=(it == NIT - 1),
                    )
                nc.vector.tensor_tensor(
                    out=v_sb[:, lt, 8 * og : 8 * og + 8, 0:HD],
                    in0=ps[:].rearrange("p (h d) -> p h d", h=8),
                    in1=bvb_sb[:, og * 512 : (og + 1) * 512].rearrange(
                        "p (h d) -> p h d", h=8
                    ),
                    op=OP.add,
                )

            bg_urgent = deque()
            bg = deque()
            _step = [0]

            def bg_pop(force=False):
                # urgent chunks drain greedily; paced chunks every other slot
                if bg_urgent:
                    bg_urgent.popleft()()
                    return
                _step[0] += 1
                if bg and (force or _step[0] % 2 == 0):
                    bg.popleft()()

            # ---- attention: one 1024-wide q chunk per head ----
            # software-pipelined across heads: head h's AV/normalize phase is
            # interleaved with head h+1's S/exp phase so PE always has S work
            # while DVE normalizes and ACT streams exps.

            def av_phase(h, p_tiles):
                po = (h % 2) * HD
                ot = h // 2
                tr_ps = shpsp.tile([HD, 1024], BF16, tag="shps")
                for qs in range(NQS):
                    ctx_ps = ctxpsp.tile([P, HD + 1], F32, tag="ctx")
                    for kt in range(NKT):
                        nc.tensor.matmul(
                            ctx_ps[:],
                            p_tiles[kt][:, qs * P : (qs + 1) * P],
                            v_sb[:, kt, h, :],
                            start=(kt == 0),
                            stop=(kt == NKT - 1),
                        )
                    den = smallp.tile([P, 1], F32, tag="den")
                    nc.vector.reciprocal(den[:], ctx_ps[:, HD : HD + 1])
                    cn = smallp.tile([P, HD], BF16, tag="cn")
                    nc.vector.tensor_scalar(
                        out=cn[:],
                        in0=ctx_ps[:, 0:HD],
                        scalar1=den[:, 0:1],
                        scalar2=None,
                        op0=OP.mult,
                    )
                    nc.tensor.transpose(
                        tr_ps[:, qs * P : (qs + 1) * P], cn[:], ident[:]
                    )
                    bg_pop()
                    yield
                nc.vector.tensor_copy(
                    out=ctxT_sb[po : po + HD, ot, :], in_=tr_ps[:]
                )

            def run_interleaved(gens_weights):
                """Round-robin generators: (gen, steps_per_turn)."""
                live = [[g, w] for g, w in gens_weights]
                while live:
                    for gw in list(live):
                        g, w = gw
                        for _ in range(w):
                            try:
                                next(g)
                            except StopIteration:
                                live.remove(gw)
                                break

            # ---- startup: eagerly project what head 0 needs first ----
            qt0, ktl0 = alloc_qk(0)
            for lc in range(2):
                qk_chunk(wq0, 0, bq_sb, qt0, lc)
            for lc in range(4):
                qk_chunk(wk0, 0, bk_sb, ktl0, lc)
            for lt in range(4):
                v_chunk(wv0, 0, lt)

            wv1 = [None]

            def queue_group(ot):
                # work queued at group ot, popped during its heads' S-steps
                if ot == 0:
                    for lt in range(4, NKT):
                        bg_urgent.append(lambda lt=lt: v_chunk(wv0, 0, lt))
                if ot < NOT - 1:
                    wtq = qk_dma(ot + 1, wqT)
                    wtk = qk_dma(ot + 1, wkT)
                    qt, ktl = alloc_qk(ot + 1)
                    for lc in range(2):
                        bg.append(
                            lambda wt=wtq, ot=ot, lc=lc, qt=qt: qk_chunk(
                                wt, ot + 1, bq_sb, qt, lc
                            )
                        )
                    for lc in range(4):
                        bg.append(
                            lambda wt=wtk, ot=ot, lc=lc, ktl=ktl: qk_chunk(
                                wt, ot + 1, bk_sb, ktl, lc
                            )
                        )
                if ot == 1:
                    wv1[0] = v_dma(1)
                if 1 <= ot <= 4:
                    for lt in range(4 * (ot - 1), 4 * ot):
                        bg.append(lambda lt=lt: v_chunk(wv1[0], 1, lt))

            # run the pipeline: S(0); then for h: interleave AV(h-1) with S(h)
            class HeadState:
                pass

            def make_s(h):
                st = HeadState()
                st.tiles = []
                po = (h % 2) * HD
                ot = h // 2

                def gen():
                    qt, ktl = qk_tiles[ot]
                    for kt in range(NKT):
                        sps = psp.tile([P, 1024], F32, tag="sps")
                        for qh in range(2):
                            nc.tensor.matmul(
                                sps[:, qh * 512 : (qh + 1) * 512],
                                ktl[po : po + HD, kt * P : (kt + 1) * P],
                                qt[po : po + HD, qh * 512 : (qh + 1) * 512],
                                start=True,
                                stop=True,
                            )
                        pt = ptp.tile([P, 1024], BF16, tag="pt")
                        nc.scalar.activation(
                            out=pt[:],
                            in_=sps[:],
                            func=AF.Exp,
                            bias=biask_sb[:, kt : kt + 1],
                            scale=SCALE,
                        )
                        st.tiles.append(pt)
                        bg_pop()
                        yield

                st.gen = gen()
                return st

            queue_group(0)
            st = make_s(0)
            for _ in st.gen:
                pass
            for h in range(1, H):
                if h % 2 == 0:
                    queue_group(h // 2)
                st_next = make_s(h)
                run_interleaved([(av_phase(h - 1, st.tiles), 1), (st_next.gen, 2)])
                st = st_next
            for _ in av_phase(H - 1, st.tiles):
                pass
            while bg_urgent:
                bg_urgent.popleft()()
            while bg:
                bg.popleft()()

            # ---- output projection + residual + layernorm ----
            gb_sb = bigp.tile([P, 2 * D], F32, tag="xT")  # reuses xT's slot
            nc.sync.dma_start(out=gb_sb[:], in_=gbeta[:, :])
            gamb_sb = gb_sb[:, 0:D]
            betb_sb = gb_sb[:, D : 2 * D]
            wo_tiles = []
            for oc in range(2):
                wt = wsp.tile([P, NIT, 512], BF16, tag="wv")  # reuses wv slots
                nc.sync.dma_start(
                    out=wt[:],
                    in_=woT[:, oc * 512 : (oc + 1) * 512].rearrange(
                        "(t p) o -> p t o", p=P
                    ),
                )
                wo_tiles.append(wt)

            for lt in range(NLT):
                xr = iop.tile([P, D], F32, tag="xr")
                nc.sync.dma_start(out=xr[:], in_=xres[lt * P : (lt + 1) * P, :])
                y = iop.tile([P, D], F32, tag="y")
                for oc in range(2):
                    ps = shpsp.tile([P, 512], F32, tag="shps")
                    for it in range(NIT):
                        nc.tensor.matmul(
                            ps[:],
                            ctxT_sb[:, it, lt * P : (lt + 1) * P],
                            wo_tiles[oc][:, it, :],
                            start=(it == 0),
                            stop=(it == NIT - 1),
                        )
                    nc.vector.tensor_tensor(
                        out=y[:, oc * 512 : (oc + 1) * 512],
                        in0=ps[:],
                        in1=xr[:, oc * 512 : (oc + 1) * 512],
                        op=OP.add,
                    )
                stats = smallp.tile([P, 2, 6], F32, tag="stats")
                nc.vector.bn_stats(stats[:, 0, :], y[:, 0:512])
                nc.vector.bn_stats(stats[:, 1, :], y[:, 512:1024])
                mv = smallp.tile([P, 2], F32, tag="mv")
                nc.vector.bn_aggr(mv[:], stats[:])
                std = smallp.tile([P, 1], F32, tag="std")
                nc.scalar.activation(
                    out=std[:], in_=mv[:, 1:2], func=AF.Sqrt, bias=eps_sb[:, 0:1]
                )
                rstd = smallp.tile([P, 1], F32, tag="rstd")
                nc.vector.reciprocal(rstd[:], std[:])
                nmr = smallp.tile([P, 1], F32, tag="nmr")  # -mu * rstd
                nc.vector.tensor_scalar(
                    out=nmr[:],
                    in0=mv[:, 0:1],
                    scalar1=rstd[:, 0:1],
                    scalar2=-1.0,
                    op0=OP.mult,
                    op1=OP.mult,
                )
                yn = iop.tile([P, D], F32, tag="xr")
                # (y - mu) * rstd on the (otherwise idle) scalar engine
                nc.scalar.activation(
                    out=yn[:],
                    in_=y[:],
                    func=AF.Identity,
                    bias=nmr[:, 0:1],
                    scale=rstd[:, 0:1],
                )
                o_sb = iop.tile([P, D], F32, tag="y")
                nc.vector.tensor_tensor(out=o_sb[:], in0=yn[:], in1=gamb_sb, op=OP.mult)
                nc.gpsimd.tensor_tensor(out=o_sb[:], in0=o_sb[:], in1=betb_sb, op=OP.add)
                nc.sync.dma_start(out=out[lt * P : (lt + 1) * P, :], in_=o_sb[:])

    nc.compile()
    return nc


def host_prep(inputs):
    """Shard + lay out the full inputs into 8 per-core in_maps."""
    bf16 = ml_dtypes.bfloat16
    x = np.asarray(inputs["x"], dtype=np.float32)
    bi = np.asarray(inputs["band_importance"], dtype=np.float32)[0]
    idx = np.argpartition(-bi, TOPK)[:TOPK]  # top-k of softmax == top-k of logits
    bias_vec = np.zeros(L, np.float32)
    bias_vec[idx] = MASK_BIAS

    wqTn = np.ascontiguousarray(np.asarray(inputs["Wq"], np.float32).T).astype(bf16)
    wkTn = np.ascontiguousarray(np.asarray(inputs["Wk"], np.float32).T).astype(bf16)
    wvTn = np.ascontiguousarray(np.asarray(inputs["Wv"], np.float32).T).astype(bf16)
    woTn = np.ascontiguousarray(np.asarray(inputs["Wo"], np.float32).T).astype(bf16)
    bq = np.asarray(inputs["bq"], np.float32).reshape(NOT, P).T
    bk = np.asarray(inputs["bk"], np.float32).reshape(NOT, P).T
    bv = np.asarray(inputs["bv"], np.float32)
    bo = np.asarray(inputs["bo"], np.float32)
    gam = np.asarray(inputs["gamma"], np.float32)
    bet = np.asarray(inputs["beta"], np.float32)
    bvb = np.ascontiguousarray(np.broadcast_to(bv, (P, D))).astype(bf16)
    gbeta = np.ascontiguousarray(
        np.concatenate(
            [np.broadcast_to(gam, (P, D)), np.broadcast_to(bet, (P, D))], axis=1
        )
    )

    in_maps = []
    for c in range(NCORES):
        b, hh = c // 2, c % 2
        own = slice(hh * LQ, (hh + 1) * LQ)
        oth = slice((1 - hh) * LQ, (2 - hh) * LQ)
        xTb = x[b].T  # [D, L] view
        xT_c = np.concatenate([xTb[:, own], xTb[:, oth]], axis=1).astype(bf16)
        pb = np.concatenate([bias_vec[own], bias_vec[oth]])
        biask_c = pb.reshape(NKT, P).T
        cpack_c = np.ascontiguousarray(
            np.concatenate([biask_c, bq, bk], axis=1), dtype=np.float32
        )
        xres_c = np.ascontiguousarray(x[b, own]) + bo[None, :]
        in_maps.append(
            {
                "xT": xT_c,
                "xres": xres_c,
                "wqT": wqTn,
                "wkT": wkTn,
                "wvT": wvTn,
                "woT": woTn,
                "cpack": cpack_c,
                "bvb": bvb,
                "gbeta": gbeta,
            }
        )
    return in_maps


def assemble(results):
    out = np.empty((B, L, D), np.float32)
    for c in range(NCORES):
        b, hh = c // 2, c % 2
        out[b, hh * LQ : (hh + 1) * LQ, :] = results[c]["out"]
    return out


_NC_CACHE = None


def kernel(**inputs):
    global _NC_CACHE
    if _NC_CACHE is None:
        _NC_CACHE = build_nc()
    in_maps = host_prep(inputs)
    res = run_bass_kernel_spmd(_NC_CACHE, in_maps, core_ids=list(range(NCORES)))
    return assemble(res.results)


# revision 69
# speedup vs baseline: 8281.8340x; 1.1615x over previous
"""Trainium2 Bass kernel for AdaptivePhysicallyConstrainedAttention.

Model (see problem reference): top-k-masked dense attention + residual + LayerNorm.
  mask  = top-3 columns of softmax(band_importance) -> additive -inf bias
  q,k,v = x @ W{q,k,v}.T + b        (B=4, L=2048, D=1024, H=16, hd=64)
  attn  = softmax(q k^T / 8 + bias) v ;  out = LN(x + attn @ Wo.T + bo) * gamma + beta

Sharding: 8 cores = (batch 4) x (query-halves 2). Each core computes K/V for its
full batch (duplicated within the pair) and attends its 1024 query rows — no
collectives. Host prep: top-k mask (tiny), weight transposes, bf16 casts, and a
per-core column permutation of x^T so every core's own query rows sit first
(keeps the graph SPMD-uniform).

On-device structure:
  - scores computed transposed (S^T = K Q^T) so the column mask is a
    per-partition activation bias and exp output feeds the AV matmul as lhsT
  - exp runs 1024-wide out of two PSUM banks (ScalarE is the critical engine;
    wide activations amortize its access latency)
  - V stored [k, head, 65] with a ones column -> AV matmul also produces the
    softmax denominator; normalization is a per-partition scale afterwards
  - all projection work is emitted as background chunks popped between
    attention S-steps so ScalarE never starves
  - matmuls in bf16 (fp32 accumulation), everything else fp32
"""

import sys

if "/opt/trn_rl_repo" not in sys.path:
    sys.path.insert(0, "/opt/trn_rl_repo")

from collections import deque

import numpy as np
import ml_dtypes

import concourse.bass as bass  # noqa: F401  (registers engines)
import concourse.tile as tile
from concourse import bacc, mybir
from concourse.bass_utils import run_bass_kernel_spmd
from concourse.masks import make_identity

BF16 = mybir.dt.bfloat16
FP8 = mybir.dt.float8e4
F32 = mybir.dt.float32
AF = mybir.ActivationFunctionType
OP = mybir.AluOpType

B, L, D, H, HD = 4, 2048, 1024, 16, 64
LQ = L // 2  # query rows per core
P = 128
NCORES = 8
TOPK = 3
SCALE = 1.0 / 8.0
MASK_BIAS = -10000.0
LN_EPS = 1e-5

NIT = D // P        # 8   contraction tiles over D
NOT = D // P        # 8   output tiles over D
NKT = L // P        # 16  key tiles
NQS = LQ // P       # 8   query subtiles
NLT = LQ // P       # 8   own-row tiles


def build_nc():
    nc = bacc.Bacc(None, target_bir_lowering=False, debug=False)

    xT = nc.declare_dram_parameter("xT", [D, L], BF16, isOutput=False)
    xres = nc.declare_dram_parameter("xres", [LQ, D], F32, isOutput=False)
    wqT = nc.declare_dram_parameter("wqT", [D, D], BF16, isOutput=False)
    wkT = nc.declare_dram_parameter("wkT", [D, D], BF16, isOutput=False)
    wvT = nc.declare_dram_parameter("wvT", [D, D], BF16, isOutput=False)
    woT = nc.declare_dram_parameter("woT", [D, D], BF16, isOutput=False)
    # packed small consts: cols 0:16 bias_k, 16:24 bq, 24:32 bk
    cpack = nc.declare_dram_parameter("cpack", [P, 32], F32, isOutput=False)
    bvb = nc.declare_dram_parameter("bvb", [P, D], BF16, isOutput=False)
    # packed gamma/beta broadcast: cols 0:D gamma, D:2D beta
    gbeta = nc.declare_dram_parameter("gbeta", [P, 2 * D], F32, isOutput=False)
    out = nc.declare_dram_parameter("out", [LQ, D], F32, isOutput=True)

    with tile.TileContext(nc) as tc:
        with (
            tc.tile_pool(name="const", bufs=1) as constp,
            tc.tile_pool(name="big", bufs=1) as bigp,
            tc.tile_pool(name="wstream", bufs=2) as wsp,
            tc.tile_pool(name="ps", bufs=2, space="PSUM") as psp,
            tc.tile_pool(name="shps", bufs=2, space="PSUM") as shpsp,
            tc.tile_pool(name="ctxps", bufs=2, space="PSUM") as ctxpsp,
            tc.tile_pool(name="pt", bufs=34) as ptp,
            tc.tile_pool(name="qkstr", bufs=1) as qkp,
            tc.tile_pool(name="small", bufs=4) as smallp,
            tc.tile_pool(name="io", bufs=2) as iop,
        ):
            # ---- resident tensors; DMAs emitted in startup-priority order ----
            xT_sb = bigp.tile([P, NIT, L], BF16, tag="xT")
            for th in range(2):
                nc.sync.dma_start(
                    out=xT_sb[:, 4 * th : 4 * th + 4, 0:512],
                    in_=xT[512 * th : 512 * (th + 1), 0:512].rearrange(
                        "(t p) l -> p t l", p=P
                    ),
                )

            def qk_dma(ot, w_dram):
                wt = wsp.tile([P, NIT, P], BF16, tag="wqk")
                nc.sync.dma_start(
                    out=wt[:],
                    in_=w_dram[:, ot * P : (ot + 1) * P].rearrange(
                        "(t p) o -> p t o", p=P
                    ),
                )
                return wt

            wq0 = qk_dma(0, wqT)
            wk0 = qk_dma(0, wkT)
            cp_sb = constp.tile([P, 32], F32, tag="cpack")
            nc.sync.dma_start(out=cp_sb[:], in_=cpack[:, :])
            biask_sb = cp_sb[:, 0:16]
            bq_sb = cp_sb[:, 16:24]
            bk_sb = cp_sb[:, 24:32]
            for lch in range(1, 4):
                nc.sync.dma_start(
                    out=xT_sb[:, :, lch * 512 : (lch + 1) * 512],
                    in_=xT[:, lch * 512 : (lch + 1) * 512].rearrange(
                        "(t p) l -> p t l", p=P
                    ),
                )

            def v_dma(og):
                wt = wsp.tile([P, NIT, 512], BF16, tag="wv")
                nc.sync.dma_start(
                    out=wt[:],
                    in_=wvT[:, og * 512 : (og + 1) * 512].rearrange(
                        "(t p) o -> p t o", p=P
                    ),
                )
                return wt

            wv0 = v_dma(0)
            bvb_sb = constp.tile([P, D], BF16, tag="bvb")
            nc.sync.dma_start(out=bvb_sb[:], in_=bvb[:, :])

            eps_sb = constp.tile([P, 1], F32, tag="eps")
            nc.vector.memset(eps_sb[:], LN_EPS)
            warm = constp.tile([P, 1], F32, tag="warm")
            nc.scalar.activation(out=warm[:], in_=eps_sb[:], func=AF.Exp)
            ident = constp.tile([P, P], BF16, tag="ident")
            make_identity(nc, ident[:])

            v_sb = bigp.tile([P, NKT, H, HD + 1], BF16, tag="v")
            ctxT_sb = bigp.tile([P, NIT, LQ], BF16, tag="ctxT")

            # ones column of the augmented V (softmax denominator trick)
            nc.vector.memset(v_sb[:, :, :, HD : HD + 1], 1.0)

            # ---- projection chunk emitters (each ~1.7us of PE work) ----
            def qk_chunk(wt, ot, bias_sb, dst_tile, lc):
                ps = shpsp.tile([P, 512], F32, tag="shps")
                for it in range(NIT):
                    nc.tensor.matmul(
                        ps[:],
                        wt[:, it, :],
                        xT_sb[:, it, lc * 512 : (lc + 1) * 512],
                        start=(it == 0),
                        stop=(it == NIT - 1),
                    )
                nc.vector.tensor_scalar(
                    out=dst_tile[:, 0, lc * 512 : (lc + 1) * 512],
                    in0=ps[:],
                    scalar1=bias_sb[:, ot : ot + 1],
                    scalar2=None,
                    op0=OP.add,
                )

            # ping-pong Q/K stream tiles with a zeroed second Ko slice:
            # DoubleRow contracts (p, i) pairs; slice i=1 stays zero so the
            # matmul adds nothing while streaming at 0.5 cycles/row
            qk8 = []
            for pp in range(2):
                qt8 = qkp.tile([P, 2, LQ], FP8, tag=f"q8{pp}")
                kt8 = qkp.tile([P, 2, L], FP8, tag=f"k8{pp}")
                nc.gpsimd.memset(qt8[:, 1, :], 0.0)
                nc.gpsimd.memset(kt8[:, 1, :], 0.0)
                qk8.append((qt8, kt8))

            qk_tiles = {}

            def alloc_qk(ot):
                qt8, kt8 = qk8[ot % 2]
                qk_tiles[ot] = (qt8, kt8)
                return qt8, kt8

            def v_chunk(wt, og, lt):
                ps = shpsp.tile([P, 512], F32, tag="shps")
                for it in range(NIT):
                    nc.tensor.matmul(
                        ps[:],
                        xT_sb[:, it, lt * P : (lt + 1) * P],
                        wt[:, it, :],
                        start=(it == 0),
                        stop=(it == NIT - 1),
                    )
                nc.vector.tensor_tensor(
                    out=v_sb[:, lt, 8 * og : 8 * og + 8, 0:HD],
                    in0=ps[:].rearrange("p (h d) -> p h d", h=8),
                    in1=bvb_sb[:, og * 512 : (og + 1) * 512].rearrange(
                        "p (h d) -> p h d", h=8
                    ),
                    op=OP.add,
                )

            bg_urgent = deque()
            bg = deque()
            _step = [0]

            def bg_pop(force=False):
                # urgent chunks drain greedily; paced chunks every other slot
                if bg_urgent:
                    bg_urgent.popleft()()
                    return
                _step[0] += 1
                if bg and (force or _step[0] % 12 == 0):
                    bg.popleft()()

            # ---- attention: one 1024-wide q chunk per head ----
            # software-pipelined across heads: head h's AV/normalize phase is
            # interleaved with head h+1's S/exp phase so PE always has S work
            # while DVE normalizes and ACT streams exps.

            def av_phase(h, p_tiles, q0=0, qw=LQ):
                po = (h % 2) * HD
                ot = h // 2
                tr_ps = shpsp.tile([HD, qw], BF16, tag="shps")
                for qs in range(qw // P):
                    ctx_ps = ctxpsp.tile([P, HD + 1], F32, tag="ctx")
                    for kt in range(NKT):
                        nc.tensor.matmul(
                            ctx_ps[:],
                            p_tiles[kt][:, qs * P : (qs + 1) * P],
                            v_sb[:, kt, h, :],
                            start=(kt == 0),
                            stop=(kt == NKT - 1),
                        )
                    den = smallp.tile([P, 1], F32, tag="den")
                    nc.vector.reciprocal(den[:], ctx_ps[:, HD : HD + 1])
                    cn = smallp.tile([P, HD], BF16, tag="cn")
                    nc.vector.tensor_scalar(
                        out=cn[:],
                        in0=ctx_ps[:, 0:HD],
                        scalar1=den[:, 0:1],
                        scalar2=None,
                        op0=OP.mult,
                    )
                    nc.tensor.transpose(
                        tr_ps[:, qs * P : (qs + 1) * P], cn[:], ident[:]
                    )
                    bg_pop()
                    yield
                nc.vector.tensor_copy(
                    out=ctxT_sb[po : po + HD, ot, q0 : q0 + qw], in_=tr_ps[:]
                )

            def run_interleaved(gens_weights):
                """Round-robin generators: (gen, steps_per_turn)."""
                live = [[g, w] for g, w in gens_weights]
                while live:
                    for gw in list(live):
                        g, w = gw
                        for _ in range(w):
                            try:
                                next(g)
                            except StopIteration:
                                live.remove(gw)
                                break

            # ---- startup: eagerly project what head 0 needs first ----
            qt0, ktl0 = alloc_qk(0)
            for lc in range(2):
                qk_chunk(wq0, 0, bq_sb, qt0, lc)
            for lc in range(4):
                qk_chunk(wk0, 0, bk_sb, ktl0, lc)
            for lt in range(4):
                v_chunk(wv0, 0, lt)

            wv1 = [None]

            def queue_group(ot):
                # work queued at group ot, popped during its heads' S-steps
                if ot == 0:
                    for lt in range(4, NKT):
                        bg_urgent.append(lambda lt=lt: v_chunk(wv0, 0, lt))
                if ot < NOT - 1:
                    wtq = qk_dma(ot + 1, wqT)
                    wtk = qk_dma(ot + 1, wkT)
                    qt, ktl = alloc_qk(ot + 1)
                    for lc in range(2):
                        bg.append(
                            lambda wt=wtq, ot=ot, lc=lc, qt=qt: qk_chunk(
                                wt, ot + 1, bq_sb, qt, lc
                            )
                        )
                    for lc in range(4):
                        bg.append(
                            lambda wt=wtk, ot=ot, lc=lc, ktl=ktl: qk_chunk(
                                wt, ot + 1, bk_sb, ktl, lc
                            )
                        )
                if ot == 1:
                    wv1[0] = v_dma(1)
                if 1 <= ot <= 4:
                    for lt in range(4 * (ot - 1), 4 * ot):
                        bg_urgent.append(lambda lt=lt: v_chunk(wv1[0], 1, lt))

            # run the pipeline: S(0); then for h: interleave AV(h-1) with S(h)
            class HeadState:
                pass

            def make_s(h, q0=0, qw=LQ):
                st = HeadState()
                st.tiles = []
                po = (h % 2) * HD
                ot = h // 2

                def gen():
                    qt8, kt8 = qk_tiles[ot]
                    for kt in range(NKT):
                        sps = psp.tile([P, qw], F32, tag="sps")
                        for qh in range(qw // 512):
                            nc.tensor.matmul(
                                sps[:, qh * 512 : (qh + 1) * 512],
                                kt8[po : po + HD, :, kt * P : (kt + 1) * P],
                                qt8[
                                    po : po + HD, :,
                                    q0 + qh * 512 : q0 + (qh + 1) * 512,
                                ],
                                start=True,
                                stop=True,
                                perf_mode=mybir.MatmulPerfMode.DoubleRow,
                            )
                        pt = ptp.tile([P, qw], BF16, tag="pt")
                        nc.scalar.activation(
                            out=pt[:],
                            in_=sps[:],
                            func=AF.Exp,
                            bias=biask_sb[:, kt : kt + 1],
                            scale=SCALE,
                        )
                        st.tiles.append(pt)
                        bg_pop()
                        yield

                st.gen = gen()
                return st

            # ---- output projection + residual + layernorm ----
            def p3_setup():
                gb_sb = bigp.tile([P, 2 * D], F32, tag="xT")  # reuses xT's slot
                nc.sync.dma_start(out=gb_sb[:], in_=gbeta[:, :])
                wo_tiles = []
                for oc in range(2):
                    wt = wsp.tile([P, NIT, 512], BF16, tag="wv")  # reuses wv slots
                    nc.sync.dma_start(
                        out=wt[:],
                        in_=woT[:, oc * 512 : (oc + 1) * 512].rearrange(
                            "(t p) o -> p t o", p=P
                        ),
                    )
                    wo_tiles.append(wt)
                return gb_sb[:, 0:D], gb_sb[:, D : 2 * D], wo_tiles

            def p3_gen(lts, gamb_sb, betb_sb, wo_tiles):
                for lt in lts:
                    xr = iop.tile([P, D], F32, tag="xr")
                    nc.sync.dma_start(
                        out=xr[:], in_=xres[lt * P : (lt + 1) * P, :]
                    )
                    y = iop.tile([P, D], F32, tag="y")
                    for oc in range(2):
                        ps = shpsp.tile([P, 512], F32, tag="shps")
                        for it in range(NIT):
                            nc.tensor.matmul(
                                ps[:],
                                ctxT_sb[:, it, lt * P : (lt + 1) * P],
                                wo_tiles[oc][:, it, :],
                                start=(it == 0),
                                stop=(it == NIT - 1),
                            )
                        nc.vector.tensor_tensor(
                            out=y[:, oc * 512 : (oc + 1) * 512],
                            in0=ps[:],
                            in1=xr[:, oc * 512 : (oc + 1) * 512],
                            op=OP.add,
                        )
                        yield
                    stats = smallp.tile([P, 2, 6], F32, tag="stats")
                    nc.vector.bn_stats(stats[:, 0, :], y[:, 0:512])
                    nc.vector.bn_stats(stats[:, 1, :], y[:, 512:1024])
                    mv = smallp.tile([P, 2], F32, tag="mv")
                    nc.vector.bn_aggr(mv[:], stats[:])
                    std = smallp.tile([P, 1], F32, tag="std")
                    nc.scalar.activation(
                        out=std[:], in_=mv[:, 1:2], func=AF.Sqrt,
                        bias=eps_sb[:, 0:1],
                    )
                    rstd = smallp.tile([P, 1], F32, tag="rstd")
                    nc.vector.reciprocal(rstd[:], std[:])
                    nmr = smallp.tile([P, 1], F32, tag="nmr")  # -mu * rstd
                    nc.vector.tensor_scalar(
                        out=nmr[:],
                        in0=mv[:, 0:1],
                        scalar1=rstd[:, 0:1],
                        scalar2=-1.0,
                        op0=OP.mult,
                        op1=OP.mult,
                    )
                    yn = iop.tile([P, D], F32, tag="xr")
                    # (y - mu) * rstd on the (otherwise idle) scalar engine
                    nc.scalar.activation(
                        out=yn[:],
                        in_=y[:],
                        func=AF.Identity,
                        bias=nmr[:, 0:1],
                        scale=rstd[:, 0:1],
                    )
                    o_sb = iop.tile([P, D], F32, tag="y")
                    # gamma*yn + beta: low half on DVE, high half on gpsimd
                    nc.vector.tensor_tensor(
                        out=o_sb[:, 0:512], in0=yn[:, 0:512],
                        in1=gamb_sb[:, 0:512], op=OP.mult,
                    )
                    nc.vector.tensor_tensor(
                        out=o_sb[:, 0:512], in0=o_sb[:, 0:512],
                        in1=betb_sb[:, 0:512], op=OP.add,
                    )
                    nc.gpsimd.tensor_tensor(
                        out=o_sb[:, 512:D], in0=yn[:, 512:D],
                        in1=gamb_sb[:, 512:D], op=OP.mult,
                    )
                    nc.gpsimd.tensor_tensor(
                        out=o_sb[:, 512:D], in0=o_sb[:, 512:D],
                        in1=betb_sb[:, 512:D], op=OP.add,
                    )
                    nc.sync.dma_start(
                        out=out[lt * P : (lt + 1) * P, 0:512], in_=o_sb[:, 0:512]
                    )
                    nc.sync.dma_start(
                        out=out[lt * P : (lt + 1) * P, 512:D], in_=o_sb[:, 512:D]
                    )
                    yield

            queue_group(0)
            st = make_s(0)
            for _ in st.gen:
                pass
            for h in range(1, H):
                if h % 2 == 0:
                    # deadline: previous group's Q/K chunks must be emitted
                    # before this group's S-phase reads them
                    while bg:
                        bg.popleft()()
                    queue_group(h // 2)
                st_next = make_s(h)
                run_interleaved([(av_phase(h - 1, st.tiles), 1), (st_next.gen, 3)])
                st = st_next
            for _ in av_phase(H - 1, st.tiles):
                pass
            while bg_urgent:
                bg_urgent.popleft()()
            while bg:
                bg.popleft()()
            gamb_sb, betb_sb, wo_tiles = p3_setup()
            for _ in p3_gen(range(NLT), gamb_sb, betb_sb, wo_tiles):
                pass

    nc.compile()
    return nc


def host_prep(inputs):
    """Shard + lay out the full inputs into 8 per-core in_maps."""
    bf16 = ml_dtypes.bfloat16
    x = np.asarray(inputs["x"], dtype=np.float32)
    bi = np.asarray(inputs["band_importance"], dtype=np.float32)[0]
    idx = np.argpartition(-bi, TOPK)[:TOPK]  # top-k of softmax == top-k of logits
    bias_vec = np.zeros(L, np.float32)
    bias_vec[idx] = MASK_BIAS

    wqTn = np.ascontiguousarray(np.asarray(inputs["Wq"], np.float32).T).astype(bf16)
    wkTn = np.ascontiguousarray(np.asarray(inputs["Wk"], np.float32).T).astype(bf16)
    wvTn = np.ascontiguousarray(np.asarray(inputs["Wv"], np.float32).T).astype(bf16)
    woTn = np.ascontiguousarray(np.asarray(inputs["Wo"], np.float32).T).astype(bf16)
    bq = np.asarray(inputs["bq"], np.float32).reshape(NOT, P).T
    bk = np.asarray(inputs["bk"], np.float32).reshape(NOT, P).T
    bv = np.asarray(inputs["bv"], np.float32)
    bo = np.asarray(inputs["bo"], np.float32)
    gam = np.asarray(inputs["gamma"], np.float32)
    bet = np.asarray(inputs["beta"], np.float32)
    bvb = np.ascontiguousarray(np.broadcast_to(bv, (P, D))).astype(bf16)
    gbeta = np.ascontiguousarray(
        np.concatenate(
            [np.broadcast_to(gam, (P, D)), np.broadcast_to(bet, (P, D))], axis=1
        )
    )

    in_maps = []
    for c in range(NCORES):
        b, hh = c // 2, c % 2
        own = slice(hh * LQ, (hh + 1) * LQ)
        oth = slice((1 - hh) * LQ, (2 - hh) * LQ)
        xTb = x[b].T  # [D, L] view
        xT_c = np.concatenate([xTb[:, own], xTb[:, oth]], axis=1).astype(bf16)
        pb = np.concatenate([bias_vec[own], bias_vec[oth]])
        biask_c = pb.reshape(NKT, P).T
        cpack_c = np.ascontiguousarray(
            np.concatenate([biask_c, bq, bk], axis=1), dtype=np.float32
        )
        xres_c = np.ascontiguousarray(x[b, own]) + bo[None, :]
        in_maps.append(
            {
                "xT": xT_c,
                "xres": xres_c,
                "wqT": wqTn,
                "wkT": wkTn,
                "wvT": wvTn,
                "woT": woTn,
                "cpack": cpack_c,
                "bvb": bvb,
                "gbeta": gbeta,
            }
        )
    return in_maps


def assemble(results):
    out = np.empty((B, L, D), np.float32)
    for c in range(NCORES):
        b, hh = c // 2, c % 2
        out[b, hh * LQ : (hh + 1) * LQ, :] = results[c]["out"]
    return out


_NC_CACHE = None


def kernel(**inputs):
    global _NC_CACHE
    if _NC_CACHE is None:
        _NC_CACHE = build_nc()
    in_maps = host_prep(inputs)
    res = run_bass_kernel_spmd(_NC_CACHE, in_maps, core_ids=list(range(NCORES)))
    return assemble(res.results)


# revision 74
# speedup vs baseline: 8336.1339x; 1.0066x over previous
"""Trainium2 Bass kernel for AdaptivePhysicallyConstrainedAttention.

Model (see problem reference): top-k-masked dense attention + residual + LayerNorm.
  mask  = top-3 columns of softmax(band_importance) -> additive -inf bias
  q,k,v = x @ W{q,k,v}.T + b        (B=4, L=2048, D=1024, H=16, hd=64)
  attn  = softmax(q k^T / 8 + bias) v ;  out = LN(x + attn @ Wo.T + bo) * gamma + beta

Sharding: 8 cores = (batch 4) x (query-halves 2). Each core computes K/V for its
full batch (duplicated within the pair) and attends its 1024 query rows — no
collectives. Host prep: top-k mask (tiny), weight transposes, bf16 casts, and a
per-core column permutation of x^T so every core's own query rows sit first
(keeps the graph SPMD-uniform).

On-device structure:
  - scores computed transposed (S^T = K Q^T) so the column mask is a
    per-partition activation bias and exp output feeds the AV matmul as lhsT
  - exp runs 1024-wide out of two PSUM banks (ScalarE is the critical engine;
    wide activations amortize its access latency)
  - V stored [k, head, 65] with a ones column -> AV matmul also produces the
    softmax denominator; normalization is a per-partition scale afterwards
  - all projection work is emitted as background chunks popped between
    attention S-steps so ScalarE never starves
  - matmuls in bf16 (fp32 accumulation), everything else fp32
"""

import sys

if "/opt/trn_rl_repo" not in sys.path:
    sys.path.insert(0, "/opt/trn_rl_repo")

from collections import deque

import numpy as np
import ml_dtypes

import concourse.bass as bass  # noqa: F401  (registers engines)
import concourse.tile as tile
from concourse import bacc, mybir
from concourse.bass_utils import run_bass_kernel_spmd
from concourse.masks import make_identity

BF16 = mybir.dt.bfloat16
FP8 = mybir.dt.float8e4
F32 = mybir.dt.float32
AF = mybir.ActivationFunctionType
OP = mybir.AluOpType

B, L, D, H, HD = 4, 2048, 1024, 16, 64
LQ = L // 2  # query rows per core
P = 128
NCORES = 8
TOPK = 3
SCALE = 1.0 / 8.0
MASK_BIAS = -10000.0
LN_EPS = 1e-5

NIT = D // P        # 8   contraction tiles over D
NOT = D // P        # 8   output tiles over D
NKT = L // P        # 16  key tiles
NQS = LQ // P       # 8   query subtiles
NLT = LQ // P       # 8   own-row tiles


def build_nc():
    nc = bacc.Bacc(None, target_bir_lowering=False, debug=False)

    xT = nc.declare_dram_parameter("xT", [D, L], BF16, isOutput=False)
    xT8 = nc.declare_dram_parameter("xT8", [D, L], FP8, isOutput=False)
    xres = nc.declare_dram_parameter("xres", [LQ, D], F32, isOutput=False)
    wqT = nc.declare_dram_parameter("wqT", [D, D], FP8, isOutput=False)
    wkT = nc.declare_dram_parameter("wkT", [D, D], FP8, isOutput=False)
    wvT = nc.declare_dram_parameter("wvT", [D, D], BF16, isOutput=False)
    woT = nc.declare_dram_parameter("woT", [D, D], BF16, isOutput=False)
    # packed small consts: cols 0:16 bias_k, 16:24 bq, 24:32 bk
    cpack = nc.declare_dram_parameter("cpack", [P, 32], F32, isOutput=False)
    bvb = nc.declare_dram_parameter("bvb", [P, D], BF16, isOutput=False)
    # packed gamma/beta broadcast: cols 0:D gamma, D:2D beta
    gbeta = nc.declare_dram_parameter("gbeta", [P, 2 * D], F32, isOutput=False)
    out = nc.declare_dram_parameter("out", [LQ, D], F32, isOutput=True)

    with tile.TileContext(nc) as tc:
        with (
            tc.tile_pool(name="const", bufs=1) as constp,
            tc.tile_pool(name="big", bufs=1) as bigp,
            tc.tile_pool(name="wstream", bufs=2) as wsp,
            tc.tile_pool(name="ps", bufs=2, space="PSUM") as psp,
            tc.tile_pool(name="shps", bufs=2, space="PSUM") as shpsp,
            tc.tile_pool(name="ctxps", bufs=2, space="PSUM") as ctxpsp,
            tc.tile_pool(name="pt", bufs=27) as ptp,
            tc.tile_pool(name="qkstr", bufs=1) as qkp,
            tc.tile_pool(name="small", bufs=4) as smallp,
            tc.tile_pool(name="io", bufs=2) as iop,
        ):
            # ---- resident tensors; DMAs emitted in startup-priority order ----
            xT_sb = bigp.tile([P, NIT, L], BF16, tag="xT")
            xT8_sb = bigp.tile([P, NIT, L], FP8, tag="xT8")
            nc.sync.dma_start(
                out=xT8_sb[:, :, 0:512],
                in_=xT8[:, 0:512].rearrange("(t p) l -> p t l", p=P),
            )
            for th in range(2):
                nc.sync.dma_start(
                    out=xT_sb[:, 4 * th : 4 * th + 4, 0:512],
                    in_=xT[512 * th : 512 * (th + 1), 0:512].rearrange(
                        "(t p) l -> p t l", p=P
                    ),
                )

            def qk_dma(ot, w_dram):
                wt = wsp.tile([P, NIT, P], FP8, tag="wqk")
                nc.sync.dma_start(
                    out=wt[:],
                    in_=w_dram[:, ot * P : (ot + 1) * P].rearrange(
                        "(t p) o -> p t o", p=P
                    ),
                )
                return wt

            wq0 = qk_dma(0, wqT)
            wk0 = qk_dma(0, wkT)
            cp_sb = constp.tile([P, 32], F32, tag="cpack")
            nc.sync.dma_start(out=cp_sb[:], in_=cpack[:, :])
            biask_sb = cp_sb[:, 0:16]
            bq_sb = cp_sb[:, 16:24]
            bk_sb = cp_sb[:, 24:32]
            for lch in range(1, 4):
                nc.sync.dma_start(
                    out=xT8_sb[:, :, lch * 512 : (lch + 1) * 512],
                    in_=xT8[:, lch * 512 : (lch + 1) * 512].rearrange(
                        "(t p) l -> p t l", p=P
                    ),
                )
                nc.sync.dma_start(
                    out=xT_sb[:, :, lch * 512 : (lch + 1) * 512],
                    in_=xT[:, lch * 512 : (lch + 1) * 512].rearrange(
                        "(t p) l -> p t l", p=P
                    ),
                )

            def v_dma(og):
                wt = wsp.tile([P, NIT, 512], BF16, tag="wv")
                nc.sync.dma_start(
                    out=wt[:],
                    in_=wvT[:, og * 512 : (og + 1) * 512].rearrange(
                        "(t p) o -> p t o", p=P
                    ),
                )
                return wt

            wv0 = v_dma(0)
            bvb_sb = constp.tile([P, D], BF16, tag="bvb")
            nc.sync.dma_start(out=bvb_sb[:], in_=bvb[:, :])

            eps_sb = constp.tile([P, 1], F32, tag="eps")
            nc.vector.memset(eps_sb[:], LN_EPS)
            warm = constp.tile([P, 1], F32, tag="warm")
            nc.scalar.activation(out=warm[:], in_=eps_sb[:], func=AF.Exp)
            ident = constp.tile([P, P], BF16, tag="ident")
            make_identity(nc, ident[:])

            v_sb = bigp.tile([P, NKT, H, HD + 1], BF16, tag="v")
            ctxT_sb = bigp.tile([P, NIT, LQ], BF16, tag="ctxT")

            # ones column of the augmented V (softmax denominator trick)
            nc.vector.memset(v_sb[:, :, :, HD : HD + 1], 1.0)

            # ---- projection chunk emitters (each ~1.7us of PE work) ----
            def qk_chunk(wt, ot, bias_sb, dst_tile, lc):
                ps = shpsp.tile([P, 512], F32, tag="shps")
                for i in range(NIT // 2):
                    nc.tensor.matmul(
                        ps[:],
                        wt[:, 2 * i : 2 * i + 2, :],
                        xT8_sb[:, 2 * i : 2 * i + 2, lc * 512 : (lc + 1) * 512],
                        start=(i == 0),
                        stop=(i == NIT // 2 - 1),
                        perf_mode=mybir.MatmulPerfMode.DoubleRow,
                    )
                nc.vector.tensor_scalar(
                    out=dst_tile[:, 0, lc * 512 : (lc + 1) * 512],
                    in0=ps[:],
                    scalar1=bias_sb[:, ot : ot + 1],
                    scalar2=None,
                    op0=OP.add,
                )

            # ping-pong Q/K stream tiles with a zeroed second Ko slice:
            # DoubleRow contracts (p, i) pairs; slice i=1 stays zero so the
            # matmul adds nothing while streaming at 0.5 cycles/row
            qk8 = []
            for pp in range(2):
                qt8 = qkp.tile([P, 2, LQ], FP8, tag=f"q8{pp}")
                kt8 = qkp.tile([P, 2, L], FP8, tag=f"k8{pp}")
                nc.gpsimd.memset(qt8[:, 1, :], 0.0)
                nc.gpsimd.memset(kt8[:, 1, :], 0.0)
                qk8.append((qt8, kt8))

            qk_tiles = {}

            def alloc_qk(ot):
                qt8, kt8 = qk8[ot % 2]
                qk_tiles[ot] = (qt8, kt8)
                return qt8, kt8

            def v_chunk(wt, og, lt):
                ps = shpsp.tile([P, 512], F32, tag="shps")
                for it in range(NIT):
                    nc.tensor.matmul(
                        ps[:],
                        xT_sb[:, it, lt * P : (lt + 1) * P],
                        wt[:, it, :],
                        start=(it == 0),
                        stop=(it == NIT - 1),
                    )
                nc.vector.tensor_tensor(
                    out=v_sb[:, lt, 8 * og : 8 * og + 8, 0:HD],
                    in0=ps[:].rearrange("p (h d) -> p h d", h=8),
                    in1=bvb_sb[:, og * 512 : (og + 1) * 512].rearrange(
                        "p (h d) -> p h d", h=8
                    ),
                    op=OP.add,
                )

            bg_urgent = deque()
            bg = deque()
            _step = [0]

            def bg_pop(force=False):
                # urgent chunks drain greedily; paced chunks every other slot
                if bg_urgent:
                    bg_urgent.popleft()()
                    return
                _step[0] += 1
                if bg and (force or _step[0] % 12 == 0):
                    bg.popleft()()

            # ---- attention: one 1024-wide q chunk per head ----
            # software-pipelined across heads: head h's AV/normalize phase is
            # interleaved with head h+1's S/exp phase so PE always has S work
            # while DVE normalizes and ACT streams exps.

            def av_phase(h, p_tiles, q0=0, qw=LQ):
                po = (h % 2) * HD
                ot = h // 2
                tr_ps = shpsp.tile([HD, qw], BF16, tag="shps")
                for qs in range(qw // P):
                    ctx_ps = ctxpsp.tile([P, HD + 1], F32, tag="ctx")
                    for kt in range(NKT):
                        nc.tensor.matmul(
                            ctx_ps[:],
                            p_tiles[kt][:, qs * P : (qs + 1) * P],
                            v_sb[:, kt, h, :],
                            start=(kt == 0),
                            stop=(kt == NKT - 1),
                        )
                    den = smallp.tile([P, 1], F32, tag="den")
                    nc.vector.reciprocal(den[:], ctx_ps[:, HD : HD + 1])
                    cn = smallp.tile([P, HD], BF16, tag="cn")
                    nc.vector.tensor_scalar(
                        out=cn[:],
                        in0=ctx_ps[:, 0:HD],
                        scalar1=den[:, 0:1],
                        scalar2=None,
                        op0=OP.mult,
                    )
                    nc.tensor.transpose(
                        tr_ps[:, qs * P : (qs + 1) * P], cn[:], ident[:]
                    )
                    bg_pop()
                    yield
                nc.vector.tensor_copy(
                    out=ctxT_sb[po : po + HD, ot, q0 : q0 + qw], in_=tr_ps[:]
                )

            def run_interleaved(gens_weights):
                """Round-robin generators: (gen, steps_per_turn)."""
                live = [[g, w] for g, w in gens_weights]
                while live:
                    for gw in list(live):
                        g, w = gw
                        for _ in range(w):
                            try:
                                next(g)
                            except StopIteration:
                                live.remove(gw)
                                break

            # ---- startup: eagerly project what head 0 needs first ----
            qt0, ktl0 = alloc_qk(0)
            for lc in range(2):
                qk_chunk(wq0, 0, bq_sb, qt0, lc)
            for lc in range(4):
                qk_chunk(wk0, 0, bk_sb, ktl0, lc)
            for lt in range(4):
                v_chunk(wv0, 0, lt)

            wv1 = [None]

            def queue_group(ot):
                # work queued at group ot, popped during its heads' S-steps
                if ot == 0:
                    for lt in range(4, NKT):
                        bg_urgent.append(lambda lt=lt: v_chunk(wv0, 0, lt))
                if ot < NOT - 1:
                    wtq = qk_dma(ot + 1, wqT)
                    wtk = qk_dma(ot + 1, wkT)
                    qt, ktl = alloc_qk(ot + 1)
                    for lc in range(2):
                        bg.append(
                            lambda wt=wtq, ot=ot, lc=lc, qt=qt: qk_chunk(
                                wt, ot + 1, bq_sb, qt, lc
                            )
                        )
                    for lc in range(4):
                        bg.append(
                            lambda wt=wtk, ot=ot, lc=lc, ktl=ktl: qk_chunk(
                                wt, ot + 1, bk_sb, ktl, lc
                            )
                        )
                if ot == 1:
                    wv1[0] = v_dma(1)
                if 1 <= ot <= 4:
                    for lt in range(4 * (ot - 1), 4 * ot):
                        bg_urgent.append(lambda lt=lt: v_chunk(wv1[0], 1, lt))

            # run the pipeline: S(0); then for h: interleave AV(h-1) with S(h)
            class HeadState:
                pass

            def make_s(h, q0=0, qw=LQ):
                st = HeadState()
                st.tiles = []
                po = (h % 2) * HD
                ot = h // 2

                def gen():
                    qt8, kt8 = qk_tiles[ot]
                    for kt in range(NKT):
                        sps = psp.tile([P, qw], F32, tag="sps")
                        for qh in range(qw // 512):
                            nc.tensor.matmul(
                                sps[:, qh * 512 : (qh + 1) * 512],
                                kt8[po : po + HD, :, kt * P : (kt + 1) * P],
                                qt8[
                                    po : po + HD, :,
                                    q0 + qh * 512 : q0 + (qh + 1) * 512,
                                ],
                                start=True,
                                stop=True,
                                perf_mode=mybir.MatmulPerfMode.DoubleRow,
                            )
                        pt = ptp.tile([P, qw], BF16, tag="pt")
                        nc.scalar.activation(
                            out=pt[:],
                            in_=sps[:],
                            func=AF.Exp,
                            bias=biask_sb[:, kt : kt + 1],
                            scale=SCALE,
                        )
                        st.tiles.append(pt)
                        bg_pop()
                        yield

                st.gen = gen()
                return st

            # ---- output projection + residual + layernorm ----
            def p3_setup():
                gb_sb = bigp.tile([P, 2 * D], F32, tag="xT")  # reuses xT's slot
                nc.sync.dma_start(out=gb_sb[:], in_=gbeta[:, :])
                wo_tiles = []
                for oc in range(2):
                    wt = wsp.tile([P, NIT, 512], BF16, tag="wv")  # reuses wv slots
                    nc.sync.dma_start(
                        out=wt[:],
                        in_=woT[:, oc * 512 : (oc + 1) * 512].rearrange(
                            "(t p) o -> p t o", p=P
                        ),
                    )
                    wo_tiles.append(wt)
                return gb_sb[:, 0:D], gb_sb[:, D : 2 * D], wo_tiles

            def p3_gen(lts, gamb_sb, betb_sb, wo_tiles):
                for lt in lts:
                    xr = iop.tile([P, D], F32, tag="xr")
                    nc.sync.dma_start(
                        out=xr[:], in_=xres[lt * P : (lt + 1) * P, :]
                    )
                    y = iop.tile([P, D], F32, tag="y")
                    for oc in range(2):
                        ps = shpsp.tile([P, 512], F32, tag="shps")
                        for it in range(NIT):
                            nc.tensor.matmul(
                                ps[:],
                                ctxT_sb[:, it, lt * P : (lt + 1) * P],
                                wo_tiles[oc][:, it, :],
                                start=(it == 0),
                                stop=(it == NIT - 1),
                            )
                        nc.vector.tensor_tensor(
                            out=y[:, oc * 512 : (oc + 1) * 512],
                            in0=ps[:],
                            in1=xr[:, oc * 512 : (oc + 1) * 512],
                            op=OP.add,
                        )
                        yield
                    stats = smallp.tile([P, 2, 6], F32, tag="stats")
                    nc.vector.bn_stats(stats[:, 0, :], y[:, 0:512])
                    nc.vector.bn_stats(stats[:, 1, :], y[:, 512:1024])
                    mv = smallp.tile([P, 2], F32, tag="mv")
                    nc.vector.bn_aggr(mv[:], stats[:])
                    std = smallp.tile([P, 1], F32, tag="std")
                    nc.scalar.activation(
                        out=std[:], in_=mv[:, 1:2], func=AF.Sqrt,
                        bias=eps_sb[:, 0:1],
                    )
                    rstd = smallp.tile([P, 1], F32, tag="rstd")
                    nc.vector.reciprocal(rstd[:], std[:])
                    nmr = smallp.tile([P, 1], F32, tag="nmr")  # -mu * rstd
                    nc.vector.tensor_scalar(
                        out=nmr[:],
                        in0=mv[:, 0:1],
                        scalar1=rstd[:, 0:1],
                        scalar2=-1.0,
                        op0=OP.mult,
                        op1=OP.mult,
                    )
                    yn = iop.tile([P, D], F32, tag="xr")
                    # (y - mu) * rstd on the (otherwise idle) scalar engine
                    nc.scalar.activation(
                        out=yn[:],
                        in_=y[:],
                        func=AF.Identity,
                        bias=nmr[:, 0:1],
                        scale=rstd[:, 0:1],
                    )
                    o_sb = iop.tile([P, D], F32, tag="y")
                    # gamma*yn + beta: low half on DVE, high half on gpsimd
                    nc.vector.tensor_tensor(
                        out=o_sb[:, 0:512], in0=yn[:, 0:512],
                        in1=gamb_sb[:, 0:512], op=OP.mult,
                    )
                    nc.vector.tensor_tensor(
                        out=o_sb[:, 0:512], in0=o_sb[:, 0:512],
                        in1=betb_sb[:, 0:512], op=OP.add,
                    )
                    nc.gpsimd.tensor_tensor(
                        out=o_sb[:, 512:D], in0=yn[:, 512:D],
                        in1=gamb_sb[:, 512:D], op=OP.mult,
                    )
                    nc.gpsimd.tensor_tensor(
                        out=o_sb[:, 512:D], in0=o_sb[:, 512:D],
                        in1=betb_sb[:, 512:D], op=OP.add,
                    )
                    nc.sync.dma_start(
                        out=out[lt * P : (lt + 1) * P, 0:512], in_=o_sb[:, 0:512]
                    )
                    nc.sync.dma_start(
                        out=out[lt * P : (lt + 1) * P, 512:D], in_=o_sb[:, 512:D]
                    )
                    yield

            queue_group(0)
            st = make_s(0)
            for _ in st.gen:
                pass
            for h in range(1, H):
                if h % 2 == 0:
                    # deadline: previous group's Q/K chunks must be emitted
                    # before this group's S-phase reads them
                    while bg:
                        bg.popleft()()
                    queue_group(h // 2)
                st_next = make_s(h)
                run_interleaved([(av_phase(h - 1, st.tiles), 1), (st_next.gen, 3)])
                st = st_next
            for _ in av_phase(H - 1, st.tiles):
                pass
            while bg_urgent:
                bg_urgent.popleft()()
            while bg:
                bg.popleft()()
            gamb_sb, betb_sb, wo_tiles = p3_setup()
            for _ in p3_gen(range(NLT), gamb_sb, betb_sb, wo_tiles):
                pass

    nc.compile()
    return nc


def host_prep(inputs):
    """Shard + lay out the full inputs into 8 per-core in_maps."""
    bf16 = ml_dtypes.bfloat16
    x = np.asarray(inputs["x"], dtype=np.float32)
    bi = np.asarray(inputs["band_importance"], dtype=np.float32)[0]
    idx = np.argpartition(-bi, TOPK)[:TOPK]  # top-k of softmax == top-k of logits
    bias_vec = np.zeros(L, np.float32)
    bias_vec[idx] = MASK_BIAS

    fp8 = ml_dtypes.float8_e4m3
    wqTn = np.ascontiguousarray(np.asarray(inputs["Wq"], np.float32).T).astype(fp8)
    wkTn = np.ascontiguousarray(np.asarray(inputs["Wk"], np.float32).T).astype(fp8)
    wvTn = np.ascontiguousarray(np.asarray(inputs["Wv"], np.float32).T).astype(bf16)
    woTn = np.ascontiguousarray(np.asarray(inputs["Wo"], np.float32).T).astype(bf16)
    bq = np.asarray(inputs["bq"], np.float32).reshape(NOT, P).T
    bk = np.asarray(inputs["bk"], np.float32).reshape(NOT, P).T
    bv = np.asarray(inputs["bv"], np.float32)
    bo = np.asarray(inputs["bo"], np.float32)
    gam = np.asarray(inputs["gamma"], np.float32)
    bet = np.asarray(inputs["beta"], np.float32)
    bvb = np.ascontiguousarray(np.broadcast_to(bv, (P, D))).astype(bf16)
    gbeta = np.ascontiguousarray(
        np.concatenate(
            [np.broadcast_to(gam, (P, D)), np.broadcast_to(bet, (P, D))], axis=1
        )
    )

    in_maps = []
    for c in range(NCORES):
        b, hh = c // 2, c % 2
        own = slice(hh * LQ, (hh + 1) * LQ)
        oth = slice((1 - hh) * LQ, (2 - hh) * LQ)
        xTb = x[b].T  # [D, L] view
        xT_cf = np.concatenate([xTb[:, own], xTb[:, oth]], axis=1)
        xT_c = xT_cf.astype(bf16)
        xT8_c = xT_cf.astype(fp8)
        pb = np.concatenate([bias_vec[own], bias_vec[oth]])
        biask_c = pb.reshape(NKT, P).T
        cpack_c = np.ascontiguousarray(
            np.concatenate([biask_c, bq, bk], axis=1), dtype=np.float32
        )
        xres_c = np.ascontiguousarray(x[b, own]) + bo[None, :]
        in_maps.append(
            {
                "xT": xT_c,
                "xT8": xT8_c,
                "xres": xres_c,
                "wqT": wqTn,
                "wkT": wkTn,
                "wvT": wvTn,
                "woT": woTn,
                "cpack": cpack_c,
                "bvb": bvb,
                "gbeta": gbeta,
            }
        )
    return in_maps


def assemble(results):
    out = np.empty((B, L, D), np.float32)
    for c in range(NCORES):
        b, hh = c // 2, c % 2
        out[b, hh * LQ : (hh + 1) * LQ, :] = results[c]["out"]
    return out


_NC_CACHE = None


def kernel(**inputs):
    global _NC_CACHE
    if _NC_CACHE is None:
        _NC_CACHE = build_nc()
    in_maps = host_prep(inputs)
    res = run_bass_kernel_spmd(_NC_CACHE, in_maps, core_ids=list(range(NCORES)))
    return assemble(res.results)


# revision 77
# speedup vs baseline: 8505.0646x; 1.0203x over previous
"""Trainium2 Bass kernel for AdaptivePhysicallyConstrainedAttention.

Model (see problem reference): top-k-masked dense attention + residual + LayerNorm.
  mask  = top-3 columns of softmax(band_importance) -> additive -inf bias
  q,k,v = x @ W{q,k,v}.T + b        (B=4, L=2048, D=1024, H=16, hd=64)
  attn  = softmax(q k^T / 8 + bias) v ;  out = LN(x + attn @ Wo.T + bo) * gamma + beta

Sharding: 8 cores = (batch 4) x (query-halves 2). Each core computes K/V for its
full batch (duplicated within the pair) and attends its 1024 query rows — no
collectives. Host prep: top-k mask (tiny), weight transposes, bf16 casts, and a
per-core column permutation of x^T so every core's own query rows sit first
(keeps the graph SPMD-uniform).

On-device structure:
  - scores computed transposed (S^T = K Q^T) so the column mask is a
    per-partition activation bias and exp output feeds the AV matmul as lhsT
  - exp runs 1024-wide out of two PSUM banks (ScalarE is the critical engine;
    wide activations amortize its access latency)
  - V stored [k, head, 65] with a ones column -> AV matmul also produces the
    softmax denominator; normalization is a per-partition scale afterwards
  - all projection work is emitted as background chunks popped between
    attention S-steps so ScalarE never starves
  - matmuls in bf16 (fp32 accumulation), everything else fp32
"""

import sys

if "/opt/trn_rl_repo" not in sys.path:
    sys.path.insert(0, "/opt/trn_rl_repo")

from collections import deque

import numpy as np
import ml_dtypes

import concourse.bass as bass  # noqa: F401  (registers engines)
import concourse.tile as tile
from concourse import bacc, mybir
from concourse.bass_utils import run_bass_kernel_spmd
from concourse.masks import make_identity

BF16 = mybir.dt.bfloat16
FP8 = mybir.dt.float8e4
F32 = mybir.dt.float32
AF = mybir.ActivationFunctionType
OP = mybir.AluOpType

B, L, D, H, HD = 4, 2048, 1024, 16, 64
LQ = L // 2  # query rows per core
P = 128
NCORES = 8
TOPK = 3
SCALE = 1.0 / 8.0
MASK_BIAS = -10000.0
LN_EPS = 1e-5

NIT = D // P        # 8   contraction tiles over D
NOT = D // P        # 8   output tiles over D
NKT = L // P        # 16  key tiles
NQS = LQ // P       # 8   query subtiles
NLT = LQ // P       # 8   own-row tiles


def build_nc():
    nc = bacc.Bacc(None, target_bir_lowering=False, debug=False)

    xT = nc.declare_dram_parameter("xT", [D, L], BF16, isOutput=False)
    xT8 = nc.declare_dram_parameter("xT8", [D, L], FP8, isOutput=False)
    xres = nc.declare_dram_parameter("xres", [LQ, D], F32, isOutput=False)
    wqT = nc.declare_dram_parameter("wqT", [D, D], FP8, isOutput=False)
    wkT = nc.declare_dram_parameter("wkT", [D, D], FP8, isOutput=False)
    wvT = nc.declare_dram_parameter("wvT", [D, D], BF16, isOutput=False)
    woT = nc.declare_dram_parameter("woT", [D, D], BF16, isOutput=False)
    # packed small consts: cols 0:16 bias_k, 16:24 bq, 24:32 bk
    cpack = nc.declare_dram_parameter("cpack", [P, 32], F32, isOutput=False)
    bvb = nc.declare_dram_parameter("bvb", [P, D], BF16, isOutput=False)
    # packed gamma/beta broadcast: cols 0:D gamma, D:2D beta
    gbeta = nc.declare_dram_parameter("gbeta", [P, 2 * D], F32, isOutput=False)
    out = nc.declare_dram_parameter("out", [LQ, D], F32, isOutput=True)

    with tile.TileContext(nc) as tc:
        with (
            tc.tile_pool(name="const", bufs=1) as constp,
            tc.tile_pool(name="big", bufs=1) as bigp,
            tc.tile_pool(name="wstream", bufs=2) as wsp,
            tc.tile_pool(name="ps", bufs=2, space="PSUM") as psp,
            tc.tile_pool(name="shps", bufs=2, space="PSUM") as shpsp,
            tc.tile_pool(name="ctxps", bufs=2, space="PSUM") as ctxpsp,
            tc.tile_pool(name="pt", bufs=27) as ptp,
            tc.tile_pool(name="qkstr", bufs=1) as qkp,
            tc.tile_pool(name="small", bufs=4) as smallp,
            tc.tile_pool(name="io", bufs=2) as iop,
        ):
            # ---- resident tensors; DMAs emitted in startup-priority order ----
            xT_sb = bigp.tile([P, NIT, L], BF16, tag="xT")
            xT8_sb = bigp.tile([P, NIT, L], FP8, tag="xT8")
            nc.sync.dma_start(
                out=xT8_sb[:, :, 0:512],
                in_=xT8[:, 0:512].rearrange("(t p) l -> p t l", p=P),
            )
            for th in range(2):
                nc.sync.dma_start(
                    out=xT_sb[:, 4 * th : 4 * th + 4, 0:512],
                    in_=xT[512 * th : 512 * (th + 1), 0:512].rearrange(
                        "(t p) l -> p t l", p=P
                    ),
                )

            def qk_dma(ot, w_dram):
                wt = wsp.tile([P, NIT, P], FP8, tag="wqk")
                nc.sync.dma_start(
                    out=wt[:],
                    in_=w_dram[:, ot * P : (ot + 1) * P].rearrange(
                        "(t p) o -> p t o", p=P
                    ),
                )
                return wt

            wq0 = qk_dma(0, wqT)
            wk0 = qk_dma(0, wkT)
            cp_sb = constp.tile([P, 32], F32, tag="cpack")
            nc.sync.dma_start(out=cp_sb[:], in_=cpack[:, :])
            biask_sb = cp_sb[:, 0:16]
            bq_sb = cp_sb[:, 16:24]
            bk_sb = cp_sb[:, 24:32]
            for lch in range(1, 4):
                nc.sync.dma_start(
                    out=xT8_sb[:, :, lch * 512 : (lch + 1) * 512],
                    in_=xT8[:, lch * 512 : (lch + 1) * 512].rearrange(
                        "(t p) l -> p t l", p=P
                    ),
                )
                nc.sync.dma_start(
                    out=xT_sb[:, :, lch * 512 : (lch + 1) * 512],
                    in_=xT[:, lch * 512 : (lch + 1) * 512].rearrange(
                        "(t p) l -> p t l", p=P
                    ),
                )

            def v_dma(og):
                wt = wsp.tile([P, NIT, 512], BF16, tag="wv")
                nc.sync.dma_start(
                    out=wt[:],
                    in_=wvT[:, og * 512 : (og + 1) * 512].rearrange(
                        "(t p) o -> p t o", p=P
                    ),
                )
                return wt

            wv0 = v_dma(0)
            bvb_sb = constp.tile([P, D], BF16, tag="bvb")
            nc.sync.dma_start(out=bvb_sb[:], in_=bvb[:, :])

            eps_sb = constp.tile([P, 1], F32, tag="eps")
            nc.vector.memset(eps_sb[:], LN_EPS)
            warm = constp.tile([P, 1], F32, tag="warm")
            nc.scalar.activation(out=warm[:], in_=eps_sb[:], func=AF.Exp)
            ident = constp.tile([P, P], BF16, tag="ident")
            make_identity(nc, ident[:])

            v_sb = bigp.tile([P, NKT, H, HD + 1], BF16, tag="v")
            ctxT_sb = bigp.tile([P, NIT, LQ], BF16, tag="ctxT")

            # ones column of the augmented V (softmax denominator trick)
            nc.vector.memset(v_sb[:, :, :, HD : HD + 1], 1.0)

            # ---- projection chunk emitters (each ~1.7us of PE work) ----
            def qk_chunk(wt, ot, bias_sb, dst_tile, lc):
                ps = shpsp.tile([P, 512], F32, tag="shps")
                for i in range(NIT // 2):
                    nc.tensor.matmul(
                        ps[:],
                        wt[:, 2 * i : 2 * i + 2, :],
                        xT8_sb[:, 2 * i : 2 * i + 2, lc * 512 : (lc + 1) * 512],
                        start=(i == 0),
                        stop=(i == NIT // 2 - 1),
                        perf_mode=mybir.MatmulPerfMode.DoubleRow,
                    )
                nc.vector.tensor_scalar(
                    out=dst_tile[:, 0, lc * 512 : (lc + 1) * 512],
                    in0=ps[:],
                    scalar1=bias_sb[:, ot : ot + 1],
                    scalar2=None,
                    op0=OP.add,
                )

            # ping-pong Q/K stream tiles with a zeroed second Ko slice:
            # DoubleRow contracts (p, i) pairs; slice i=1 stays zero so the
            # matmul adds nothing while streaming at 0.5 cycles/row
            qk8 = []
            for pp in range(2):
                qt8 = qkp.tile([P, 2, LQ], FP8, tag=f"q8{pp}")
                kt8 = qkp.tile([P, 2, L], FP8, tag=f"k8{pp}")
                nc.gpsimd.memset(qt8[:, 1, :], 0.0)
                nc.gpsimd.memset(kt8[:, 1, :], 0.0)
                qk8.append((qt8, kt8))

            qk_tiles = {}

            def alloc_qk(ot):
                qt8, kt8 = qk8[ot % 2]
                qk_tiles[ot] = (qt8, kt8)
                return qt8, kt8

            def v_chunk(wt, og, lt):
                ps = shpsp.tile([P, 512], F32, tag="shps")
                for it in range(NIT):
                    nc.tensor.matmul(
                        ps[:],
                        xT_sb[:, it, lt * P : (lt + 1) * P],
                        wt[:, it, :],
                        start=(it == 0),
                        stop=(it == NIT - 1),
                    )
                nc.vector.tensor_tensor(
                    out=v_sb[:, lt, 8 * og : 8 * og + 8, 0:HD],
                    in0=ps[:].rearrange("p (h d) -> p h d", h=8),
                    in1=bvb_sb[:, og * 512 : (og + 1) * 512].rearrange(
                        "p (h d) -> p h d", h=8
                    ),
                    op=OP.add,
                )

            bg_urgent = deque()
            bg = deque()
            _step = [0]

            def bg_pop(force=False):
                # urgent chunks drain greedily; paced chunks every other slot
                if bg_urgent:
                    bg_urgent.popleft()()
                    return
                _step[0] += 1
                if bg and (force or _step[0] % 8 == 0):
                    bg.popleft()()

            # ---- attention: one 1024-wide q chunk per head ----
            # software-pipelined across heads: head h's AV/normalize phase is
            # interleaved with head h+1's S/exp phase so PE always has S work
            # while DVE normalizes and ACT streams exps.

            def av_phase(h, p_tiles, q0=0, qw=LQ):
                po = (h % 2) * HD
                ot = h // 2
                tr_ps = shpsp.tile([HD, qw], BF16, tag="shps")
                for qs in range(qw // P):
                    ctx_ps = ctxpsp.tile([P, HD + 1], F32, tag="ctx")
                    for kt in range(NKT):
                        nc.tensor.matmul(
                            ctx_ps[:],
                            p_tiles[kt][:, qs * P : (qs + 1) * P],
                            v_sb[:, kt, h, :],
                            start=(kt == 0),
                            stop=(kt == NKT - 1),
                        )
                    den = smallp.tile([P, 1], F32, tag="den")
                    nc.vector.reciprocal(den[:], ctx_ps[:, HD : HD + 1])
                    cn = smallp.tile([P, HD], BF16, tag="cn")
                    nc.vector.tensor_scalar(
                        out=cn[:],
                        in0=ctx_ps[:, 0:HD],
                        scalar1=den[:, 0:1],
                        scalar2=None,
                        op0=OP.mult,
                    )
                    nc.tensor.transpose(
                        tr_ps[:, qs * P : (qs + 1) * P], cn[:], ident[:]
                    )
                    bg_pop()
                    yield
                nc.vector.tensor_copy(
                    out=ctxT_sb[po : po + HD, ot, q0 : q0 + qw], in_=tr_ps[:]
                )

            def run_interleaved(gens_weights):
                """Round-robin generators: (gen, steps_per_turn)."""
                live = [[g, w] for g, w in gens_weights]
                while live:
                    for gw in list(live):
                        g, w = gw
                        for _ in range(w):
                            try:
                                next(g)
                            except StopIteration:
                                live.remove(gw)
                                break

            # ---- startup: eagerly project what head 0 needs first ----
            qt0, ktl0 = alloc_qk(0)
            for lc in range(2):
                qk_chunk(wq0, 0, bq_sb, qt0, lc)
            for lc in range(4):
                qk_chunk(wk0, 0, bk_sb, ktl0, lc)
            for lt in range(4):
                v_chunk(wv0, 0, lt)

            wv1 = [None]

            def queue_group(ot):
                # work queued at group ot, popped during its heads' S-steps
                if ot == 0:
                    for lt in range(4, NKT):
                        bg_urgent.append(lambda lt=lt: v_chunk(wv0, 0, lt))
                if ot < NOT - 1:
                    wtq = qk_dma(ot + 1, wqT)
                    wtk = qk_dma(ot + 1, wkT)
                    qt, ktl = alloc_qk(ot + 1)
                    for lc in range(2):
                        bg.append(
                            lambda wt=wtq, ot=ot, lc=lc, qt=qt: qk_chunk(
                                wt, ot + 1, bq_sb, qt, lc
                            )
                        )
                    for lc in range(4):
                        bg.append(
                            lambda wt=wtk, ot=ot, lc=lc, ktl=ktl: qk_chunk(
                                wt, ot + 1, bk_sb, ktl, lc
                            )
                        )
                if ot == 0:
                    wv1[0] = v_dma(1)
                if 0 <= ot <= 3:
                    # paced: V chunks are the expensive filler now; the group
                    # boundary force-drain still meets av(8)'s emission deadline
                    for lt in range(4 * ot, 4 * ot + 4):
                        bg.append(lambda lt=lt: v_chunk(wv1[0], 1, lt))

            # run the pipeline: S(0); then for h: interleave AV(h-1) with S(h)
            class HeadState:
                pass

            def make_s(h, q0=0, qw=LQ):
                st = HeadState()
                st.tiles = []
                po = (h % 2) * HD
                ot = h // 2

                def gen():
                    qt8, kt8 = qk_tiles[ot]
                    for kt in range(NKT):
                        sps = psp.tile([P, qw], F32, tag="sps")
                        for qh in range(qw // 512):
                            nc.tensor.matmul(
                                sps[:, qh * 512 : (qh + 1) * 512],
                                kt8[po : po + HD, :, kt * P : (kt + 1) * P],
                                qt8[
                                    po : po + HD, :,
                                    q0 + qh * 512 : q0 + (qh + 1) * 512,
                                ],
                                start=True,
                                stop=True,
                                perf_mode=mybir.MatmulPerfMode.DoubleRow,
                            )
                        pt = ptp.tile([P, qw], BF16, tag="pt")
                        nc.scalar.activation(
                            out=pt[:],
                            in_=sps[:],
                            func=AF.Exp,
                            bias=biask_sb[:, kt : kt + 1],
                            scale=SCALE,
                        )
                        st.tiles.append(pt)
                        bg_pop()
                        yield

                st.gen = gen()
                return st

            # ---- output projection + residual + layernorm ----
            def p3_setup():
                gb_sb = bigp.tile([P, 2 * D], F32, tag="xT")  # reuses xT's slot
                nc.sync.dma_start(out=gb_sb[:], in_=gbeta[:, :])
                wo_tiles = []
                for oc in range(2):
                    wt = wsp.tile([P, NIT, 512], BF16, tag="wv")  # reuses wv slots
                    nc.sync.dma_start(
                        out=wt[:],
                        in_=woT[:, oc * 512 : (oc + 1) * 512].rearrange(
                            "(t p) o -> p t o", p=P
                        ),
                    )
                    wo_tiles.append(wt)
                return gb_sb[:, 0:D], gb_sb[:, D : 2 * D], wo_tiles

            def p3_gen(lts, gamb_sb, betb_sb, wo_tiles):
                for lt in lts:
                    xr = iop.tile([P, D], F32, tag="xr")
                    nc.sync.dma_start(
                        out=xr[:], in_=xres[lt * P : (lt + 1) * P, :]
                    )
                    y = iop.tile([P, D], F32, tag="y")
                    for oc in range(2):
                        ps = shpsp.tile([P, 512], F32, tag="shps")
                        for it in range(NIT):
                            nc.tensor.matmul(
                                ps[:],
                                ctxT_sb[:, it, lt * P : (lt + 1) * P],
                                wo_tiles[oc][:, it, :],
                                start=(it == 0),
                                stop=(it == NIT - 1),
                            )
                        nc.vector.tensor_tensor(
                            out=y[:, oc * 512 : (oc + 1) * 512],
                            in0=ps[:],
                            in1=xr[:, oc * 512 : (oc + 1) * 512],
                            op=OP.add,
                        )
                        yield
                    stats = smallp.tile([P, 2, 6], F32, tag="stats")
                    nc.vector.bn_stats(stats[:, 0, :], y[:, 0:512])
                    nc.vector.bn_stats(stats[:, 1, :], y[:, 512:1024])
                    mv = smallp.tile([P, 2], F32, tag="mv")
                    nc.vector.bn_aggr(mv[:], stats[:])
                    std = smallp.tile([P, 1], F32, tag="std")
                    nc.scalar.activation(
                        out=std[:], in_=mv[:, 1:2], func=AF.Sqrt,
                        bias=eps_sb[:, 0:1],
                    )
                    rstd = smallp.tile([P, 1], F32, tag="rstd")
                    nc.vector.reciprocal(rstd[:], std[:])
                    nmr = smallp.tile([P, 1], F32, tag="nmr")  # -mu * rstd
                    nc.vector.tensor_scalar(
                        out=nmr[:],
                        in0=mv[:, 0:1],
                        scalar1=rstd[:, 0:1],
                        scalar2=-1.0,
                        op0=OP.mult,
                        op1=OP.mult,
                    )
                    yn = iop.tile([P, D], F32, tag="xr")
                    # (y - mu) * rstd on the (otherwise idle) scalar engine
                    nc.scalar.activation(
                        out=yn[:],
                        in_=y[:],
                        func=AF.Identity,
                        bias=nmr[:, 0:1],
                        scale=rstd[:, 0:1],
                    )
                    o_sb = iop.tile([P, D], F32, tag="y")
                    # gamma*yn + beta: low half on DVE, high half on gpsimd
                    nc.vector.tensor_tensor(
                        out=o_sb[:, 0:512], in0=yn[:, 0:512],
                        in1=gamb_sb[:, 0:512], op=OP.mult,
                    )
                    nc.vector.tensor_tensor(
                        out=o_sb[:, 0:512], in0=o_sb[:, 0:512],
                        in1=betb_sb[:, 0:512], op=OP.add,
                    )
                    nc.gpsimd.tensor_tensor(
                        out=o_sb[:, 512:D], in0=yn[:, 512:D],
                        in1=gamb_sb[:, 512:D], op=OP.mult,
                    )
                    nc.gpsimd.tensor_tensor(
                        out=o_sb[:, 512:D], in0=o_sb[:, 512:D],
                        in1=betb_sb[:, 512:D], op=OP.add,
                    )
                    nc.sync.dma_start(
                        out=out[lt * P : (lt + 1) * P, 0:512], in_=o_sb[:, 0:512]
                    )
                    nc.sync.dma_start(
                        out=out[lt * P : (lt + 1) * P, 512:D], in_=o_sb[:, 512:D]
                    )
                    yield

            queue_group(0)
            st = make_s(0)
            for _ in st.gen:
                pass
            for h in range(1, H):
                if h % 2 == 0:
                    # deadline: previous group's Q/K chunks must be emitted
                    # before this group's S-phase reads them
                    while bg:
                        bg.popleft()()
                    queue_group(h // 2)
                st_next = make_s(h)
                run_interleaved([(av_phase(h - 1, st.tiles), 1), (st_next.gen, 3)])
                st = st_next
            for _ in av_phase(H - 1, st.tiles):
                pass
            while bg_urgent:
                bg_urgent.popleft()()
            while bg:
                bg.popleft()()
            gamb_sb, betb_sb, wo_tiles = p3_setup()
            for _ in p3_gen(range(NLT), gamb_sb, betb_sb, wo_tiles):
                pass

    nc.compile()
    return nc


def host_prep(inputs):
    """Shard + lay out the full inputs into 8 per-core in_maps."""
    bf16 = ml_dtypes.bfloat16
    x = np.asarray(inputs["x"], dtype=np.float32)
    bi = np.asarray(inputs["band_importance"], dtype=np.float32)[0]
    idx = np.argpartition(-bi, TOPK)[:TOPK]  # top-k of softmax == top-k of logits
    bias_vec = np.zeros(L, np.float32)
    bias_vec[idx] = MASK_BIAS

    fp8 = ml_dtypes.float8_e4m3
    wqTn = np.ascontiguousarray(np.asarray(inputs["Wq"], np.float32).T).astype(fp8)
    wkTn = np.ascontiguousarray(np.asarray(inputs["Wk"], np.float32).T).astype(fp8)
    wvTn = np.ascontiguousarray(np.asarray(inputs["Wv"], np.float32).T).astype(bf16)
    woTn = np.ascontiguousarray(np.asarray(inputs["Wo"], np.float32).T).astype(bf16)
    bq = np.asarray(inputs["bq"], np.float32).reshape(NOT, P).T
    bk = np.asarray(inputs["bk"], np.float32).reshape(NOT, P).T
    bv = np.asarray(inputs["bv"], np.float32)
    bo = np.asarray(inputs["bo"], np.float32)
    gam = np.asarray(inputs["gamma"], np.float32)
    bet = np.asarray(inputs["beta"], np.float32)
    bvb = np.ascontiguousarray(np.broadcast_to(bv, (P, D))).astype(bf16)
    gbeta = np.ascontiguousarray(
        np.concatenate(
            [np.broadcast_to(gam, (P, D)), np.broadcast_to(bet, (P, D))], axis=1
        )
    )

    in_maps = []
    for c in range(NCORES):
        b, hh = c // 2, c % 2
        own = slice(hh * LQ, (hh + 1) * LQ)
        oth = slice((1 - hh) * LQ, (2 - hh) * LQ)
        xTb = x[b].T  # [D, L] view
        xT_cf = np.concatenate([xTb[:, own], xTb[:, oth]], axis=1)
        xT_c = xT_cf.astype(bf16)
        xT8_c = xT_cf.astype(fp8)
        pb = np.concatenate([bias_vec[own], bias_vec[oth]])
        biask_c = pb.reshape(NKT, P).T
        cpack_c = np.ascontiguousarray(
            np.concatenate([biask_c, bq, bk], axis=1), dtype=np.float32
        )
        xres_c = np.ascontiguousarray(x[b, own]) + bo[None, :]
        in_maps.append(
            {
                "xT": xT_c,
                "xT8": xT8_c,
                "xres": xres_c,
                "wqT": wqTn,
                "wkT": wkTn,
                "wvT": wvTn,
                "woT": woTn,
                "cpack": cpack_c,
                "bvb": bvb,
                "gbeta": gbeta,
            }
        )
    return in_maps


def assemble(results):
    out = np.empty((B, L, D), np.float32)
    for c in range(NCORES):
        b, hh = c // 2, c % 2
        out[b, hh * LQ : (hh + 1) * LQ, :] = results[c]["out"]
    return out


_NC_CACHE = None


def kernel(**inputs):
    global _NC_CACHE
    if _NC_CACHE is None:
        _NC_CACHE = build_nc()
    in_maps = host_prep(inputs)
    res = run_bass_kernel_spmd(_NC_CACHE, in_maps, core_ids=list(range(NCORES)))
    return assemble(res.results)
